# revision 28
# baseline (speedup 1.0000x reference)
"""Trainium2 Bass kernel for nn_CL_VAE (Multi-VAE loss + contrastive-learning KL).

Strategy (8 NeuronCores, data-parallel over batch rows + common users):
  - core c owns batch rows [128c, 128c+128) and the common users whose
    batch_idx falls in that range (padded to 128 user slots, user = SBUF
    partition).
  - Host prep is layout-only: shard/transpose/concat/cast inputs, build
    index tables + 0/1 masks from the integer index inputs.
  - Device per core:
      phase 0: gather before_score at (user, inter_idx) positions with
               gpsimd ap_gather (ragged-packed word indices + parity
               select, split by item halves), w = b_sel at valid slots,
               S0 = sum w, S1 = sum w*log(b_sel).
      phase 1: one fused matmul over K = I(+1 bias row):
               [h_pre | rating@dec_w | rating@dec_b | sum(rating)] =
               rating_aug^T.T @ [enc_w^T | dec_w | dec_b | 1].
      phase 2: decoder matmuls logits = [h|1] @ [dec_w^T; dec_b] for both
               batch rows and user rows from resident weights, high item
               half first (so its user-logit gather overlaps the low
               half); exp+accumulate for softmax denominators.
      phase 3: lse = log(sum exp), gather user logits, S2 = sum(w*glog),
               combine into the two loss partials, reduce over partitions
               with a ones-matmul.
  - Host combines the 8 per-core [1,2] partials by summation only.

log-softmax identity used (exact in real arithmetic):
  sum_i log_probs*rating = sum_i logits*rating - lse*sum_i rating
  log(p_sel) = logits_sel - lse      (since p_sel = exp(log_probs))
The +EPS terms of the reference cancel in (log_b - log_p) and are below
fp32 resolution elsewhere.

Gathers run on bf16 data but ap_gather moves 4-byte words, so indices
are word indices (il>>1) and a host-provided parity mask pair selects
the bf16 half: val = ipar*even + par*odd.
"""

import numpy as np
import ml_dtypes

# ---- hardcoded problem shapes ----
B, I, D, U, L = 1024, 20000, 200, 512, 200
NCORES = 8
BSH = B // NCORES      # 128 batch rows per core
UCAP = 128             # padded user slots per core
KTOT = I + 1           # contraction length (+1 bias/ones row)
KG = 8                 # k-tiles per DMA batch in phase 1
KT = 160               # k-tiles (zero-padded up from 157)
NG = KT // KG          # 20 phase-1 DMA groups
CH = 500               # decoder free-dim chunk
NCH = I // CH          # 40
IHALF = I // 2         # item split for gather overlap (10000)
NCHH = NCH // 2        # 20 chunks per half
NIDX = 1440            # ragged-packed gather indices per 16-part group/half

_BF16 = ml_dtypes.bfloat16

_prog_cache = {}


def _build_program():
    import concourse.bacc as bacc
    import concourse.mybir as mybir
    import concourse.tile as tile
    from contextlib import ExitStack

    f32 = mybir.dt.float32
    bf16 = mybir.dt.bfloat16
    i16 = mybir.dt.int16
    Act = mybir.ActivationFunctionType
    Alu = mybir.AluOpType

    nc = bacc.Bacc("TRN2", target_bir_lowering=False, debug=False,
                   num_devices=NCORES)

    ratingT = nc.dram_tensor("ratingT", [NG, 128, KG, BSH], bf16,
                             kind="ExternalInput")
    xmat = nc.dram_tensor("xmat", [NG, 128, KG, 402], bf16,
                          kind="ExternalInput")
    dwta_d = nc.dram_tensor("dwta", [128, I], bf16, kind="ExternalInput")
    dwtb_d = nc.dram_tensor("dwtb", [73, I], bf16, kind="ExternalInput")
    bef_d = [nc.dram_tensor(f"bef_{h}", [UCAP, IHALF], bf16,
                            kind="ExternalInput") for h in range(2)]
    gidx_d = [nc.dram_tensor(f"gidx_{h}", [128, NIDX // 16], i16,
                             kind="ExternalInput") for h in range(2)]
    vm_d = [nc.dram_tensor(f"vm_{h}", [128, NIDX], bf16,
                           kind="ExternalInput") for h in range(2)]
    par_d = [nc.dram_tensor(f"par_{h}", [128, NIDX], bf16,
                            kind="ExternalInput") for h in range(2)]
    ipar_d = [nc.dram_tensor(f"ipar_{h}", [128, NIDX], bf16,
                             kind="ExternalInput") for h in range(2)]
    onehot = nc.dram_tensor("onehot", [128, UCAP], f32, kind="ExternalInput")
    ident = nc.dram_tensor("ident", [128, 128], bf16, kind="ExternalInput")
    piw_s = nc.dram_tensor("piw_s", [UCAP, 1], f32, kind="ExternalInput")
    len_s = nc.dram_tensor("len_s", [UCAP, 1], f32, kind="ExternalInput")
    out_d = nc.dram_tensor("out", [1, 2], f32, kind="ExternalOutput")

    with ExitStack() as ctx:
        tc = ctx.enter_context(tile.TileContext(nc))
        pool = ctx.enter_context(tc.tile_pool(name="main", bufs=1))
        big = ctx.enter_context(tc.tile_pool(name="big", bufs=1))
        gpool = ctx.enter_context(tc.tile_pool(name="gat", bufs=1))
        stream = ctx.enter_context(tc.tile_pool(name="stream", bufs=3))
        psum = ctx.enter_context(tc.tile_pool(name="ps", bufs=1, space="PSUM"))
        psmm = ctx.enter_context(tc.tile_pool(name="psmm", bufs=2,
                                              space="PSUM"))

        def parity_select(gw, par, ipar, out_dtype, name):
            """val = ipar*even(gw) + par*odd(gw); returns [128, NIDX] tile."""
            wb = gw[:].bitcast(bf16).rearrange("p (j t) -> p j t", t=2)
            t_e = gpool.tile([128, NIDX], out_dtype, tag="val", bufs=2,
                             name=f"{name}_e")
            nc.vector.tensor_mul(t_e[:], ipar[:], wb[:, :, 0:1])
            t_o = gpool.tile([128, NIDX], out_dtype, tag="val", bufs=2,
                             name=f"{name}_o")
            nc.vector.tensor_mul(t_o[:], par[:], wb[:, :, 1:2])
            val = gpool.tile([128, NIDX], out_dtype, tag="val2", bufs=2,
                             name=f"{name}_v")
            nc.vector.tensor_add(val[:], t_e[:], t_o[:])
            return val

        # ------- phase 1 main matmul stream (DMA priority: first) -------
        ps1 = psmm.tile([128, 402], f32, tag="mm")
        for g in range(NG):
            rt_g = stream.tile([128, KG, BSH], bf16, tag="rt", bufs=3)
            nc.scalar.dma_start(out=rt_g[:], in_=ratingT[g])
            x_g = stream.tile([128, KG, 402], bf16, tag="xk", bufs=2)
            nc.sync.dma_start(out=x_g[:], in_=xmat[g])
            for kk in range(KG):
                k = KG * g + kk
                nc.tensor.matmul(ps1[:], rt_g[:, kk, :], x_g[:, kk, :],
                                 start=(k == 0), stop=(k == KT - 1))

        # Alternate DMA issue between the two HWDGE engines so chunks
        # spread across queues and run in parallel.
        _eng = [nc.sync, nc.scalar]
        _ei = [0]

        def dma(out, in_):
            _eng[_ei[0] % 2].dma_start(out=out, in_=in_)
            _ei[0] += 1

        # ------- before_score halves (second; gathers gate phase 2) -------
        bef = []
        for h in (1, 0):
            bt = big.tile([UCAP, IHALF], bf16, tag=f"b{h}", name=f"bef{h}")
            for s in range(0, IHALF, 2500):
                dma(bt[:, s:s + 2500], bef_d[h][:, s:s + 2500])
            bef.append(bt)
        bef = bef[::-1]  # index by half again
        gidx, vm, par, ipar = [], [], [], []
        for h in range(2):
            gt = pool.tile([128, NIDX // 16], i16, name=f"gidx{h}")
            dma(gt[:], gidx_d[h][:])
            gidx.append(gt)

        # ------- resident decoder weights (third; high half first) -------
        dwta = pool.tile([128, I], bf16)
        dwtb = pool.tile([73, I], bf16)
        for s in range(IHALF, I, 2500):
            dma(dwta[:, s:s + 2500], dwta_d[:, s:s + 2500])
            dma(dwtb[:, s:s + 2500], dwtb_d[:, s:s + 2500])
        for s in range(0, IHALF, 2500):
            dma(dwta[:, s:s + 2500], dwta_d[:, s:s + 2500])
            dma(dwtb[:, s:s + 2500], dwtb_d[:, s:s + 2500])

        # ------- gather tables + small constants (fourth) -------
        for h in range(2):
            vt = pool.tile([128, NIDX], bf16, name=f"vm{h}")
            dma(vt[:], vm_d[h][:])
            vm.append(vt)
            pt = pool.tile([128, NIDX], bf16, name=f"par{h}")
            dma(pt[:], par_d[h][:])
            par.append(pt)
            it = pool.tile([128, NIDX], bf16, name=f"ipar{h}")
            dma(it[:], ipar_d[h][:])
            ipar.append(it)
        onehot_sb = pool.tile([128, UCAP], f32)
        dma(onehot_sb[:], onehot[:])
        ident_sb = pool.tile([128, 128], bf16)
        dma(ident_sb[:], ident[:])
        piw_sb = pool.tile([UCAP, 1], f32)
        dma(piw_sb[:], piw_s[:])
        len_sb = pool.tile([UCAP, 1], f32)
        dma(len_sb[:], len_s[:])

        # ------- phase 0: before_score gathers (idle gpsimd, early) -------
        s0 = pool.tile([128, 1], f32)
        s1 = pool.tile([128, 1], f32)
        w_t = [None, None]
        for h in (1, 0):
            gw = gpool.tile([128, NIDX], f32, tag="tmp", bufs=2,
                            name=f"bw{h}")
            nc.gpsimd.ap_gather(gw[:], bef[h][:].bitcast(f32), gidx[h][:],
                                channels=128, num_elems=IHALF // 2, d=1,
                                num_idxs=NIDX)
            val = parity_select(gw, par[h], ipar[h], bf16, f"bv{h}")
            logb = gpool.tile([128, NIDX], f32, tag="tmp", bufs=2,
                              name=f"logb{h}")
            nc.scalar.activation(logb[:], val[:], Act.Ln)
            wt = pool.tile([128, NIDX], f32, name=f"w{h}")
            nc.vector.tensor_mul(wt[:], vm[h][:], val[:])
            w_t[h] = wt
            s0h = pool.tile([128, 1], f32, tag="s0h", bufs=2)
            nc.vector.tensor_reduce(s0h[:], wt[:], axis=mybir.AxisListType.X,
                                    op=Alu.add)
            scr_a = gpool.tile([128, NIDX], f32, tag="tmp", bufs=2,
                               name=f"wl{h}")
            nc.vector.tensor_mul(scr_a[:], wt[:], logb[:])
            s1h = pool.tile([128, 1], f32, tag="s1h", bufs=2)
            nc.vector.tensor_reduce(s1h[:], scr_a[:],
                                    axis=mybir.AxisListType.X, op=Alu.add)
            if h == 1:
                nc.vector.tensor_copy(s0[:], s0h[:])
                nc.vector.tensor_copy(s1[:], s1h[:])
            else:
                nc.vector.tensor_add(s0[:], s0[:], s0h[:])
                nc.vector.tensor_add(s1[:], s1[:], s1h[:])

        # ------- phase 1 epilogue -------
        h_f = pool.tile([128, D], f32)
        nc.scalar.activation(h_f[:], ps1[:, 0:D], Act.Tanh)
        h_bf = pool.tile([128, D + 1], bf16)
        nc.vector.tensor_copy(h_bf[:, 0:D], h_f[:])
        nc.vector.memset(h_bf[:, D:D + 1], 1.0)
        scr200 = pool.tile([128, D], f32)
        nc.vector.tensor_mul(scr200[:], h_f[:], ps1[:, D:2 * D])
        dot_p = pool.tile([128, 1], f32)
        nc.vector.tensor_reduce(dot_p[:], scr200[:], axis=mybir.AxisListType.X,
                                op=Alu.add)
        dot_row = pool.tile([128, 1], f32)
        nc.vector.tensor_add(dot_row[:], dot_p[:], ps1[:, 2 * D:2 * D + 1])
        rsum = pool.tile([128, 1], f32)
        nc.vector.tensor_copy(rsum[:], ps1[:, 2 * D + 1:2 * D + 2])

        ps_hu = psum.tile([128, D], f32, tag="sm1")
        nc.tensor.matmul(ps_hu[:], onehot_sb[:], h_f[:], start=True, stop=True)
        hu_bf = pool.tile([128, D + 1], bf16)
        nc.vector.tensor_copy(hu_bf[:, 0:D], ps_hu[:])
        nc.vector.memset(hu_bf[:, D:D + 1], 1.0)

        # transposes: h^T and h_u^T, each augmented with a trailing ones row
        hta = pool.tile([128, BSH], bf16)
        htb = pool.tile([73, BSH], bf16)
        huta = pool.tile([128, UCAP], bf16)
        hutb = pool.tile([73, UCAP], bf16)
        for src, dsta, dstb in ((h_bf, hta, htb), (hu_bf, huta, hutb)):
            ps_t1 = psum.tile([128, 128], bf16, tag="sm2", bufs=2)
            nc.tensor.transpose(ps_t1[:], src[:, 0:128], ident_sb[:])
            nc.vector.tensor_copy(dsta[:], ps_t1[:])
            ps_t2 = psum.tile([73, 128], bf16, tag="sm2", bufs=2)
            nc.tensor.transpose(ps_t2[:], src[:, 128:D + 1], ident_sb[:])
            nc.vector.tensor_copy(dstb[:], ps_t2[:])

        # ------- phase 2: decoder stream, high item half first -------
        ulog = [None, None]
        ulog[1] = big.tile([UCAP, IHALF], bf16, tag="b1", name="ulog1")
        ulog[0] = big.tile([UCAP, IHALF], bf16, tag="b0", name="ulog0")
        ssum = pool.tile([128, NCH], f32)
        est = stream.tile([128, CH], f32, tag="est", bufs=1)
        for ci in range(NCH):
            c = ci + NCHH if ci < NCHH else ci - NCHH  # hi half first
            das = dwta[:, CH * c:CH * (c + 1)]
            dbs = dwtb[:, CH * c:CH * (c + 1)]
            pm = psmm.tile([128, CH], f32, tag="mm")
            nc.tensor.matmul(pm[:], hta[:], das, start=True, stop=False)
            nc.tensor.matmul(pm[:], htb[:], dbs, start=False, stop=True)
            pu = psmm.tile([128, CH], f32, tag="pu")
            nc.tensor.matmul(pu[:], huta[:], das, start=True, stop=False)
            nc.tensor.matmul(pu[:], hutb[:], dbs, start=False, stop=True)
            nc.scalar.activation(est[:], pm[:], Act.Exp,
                                 accum_out=ssum[:, c:c + 1])
            half = 1 if c >= NCHH else 0
            c0 = CH * (c - NCHH) if c >= NCHH else CH * c
            nc.vector.tensor_copy(ulog[half][:, c0:c0 + CH], pu[:])

        # ------- phase 3: lse, user-logit gathers, combine -------
        s2 = pool.tile([128, 1], f32)
        for h in (1, 0):
            gw = gpool.tile([128, NIDX], f32, tag="tmp", bufs=2,
                            name=f"uw{h}")
            nc.gpsimd.ap_gather(gw[:], ulog[h][:].bitcast(f32), gidx[h][:],
                                channels=128, num_elems=IHALF // 2, d=1,
                                num_idxs=NIDX)
            valg = parity_select(gw, par[h], ipar[h], f32, f"uv{h}")
            scr = gpool.tile([128, NIDX], f32, tag="tmp", bufs=2,
                             name=f"ws{h}")
            nc.vector.tensor_mul(scr[:], w_t[h][:], valg[:])
            s2h = pool.tile([128, 1], f32, tag="s2h", bufs=2)
            nc.vector.tensor_reduce(s2h[:], scr[:], axis=mybir.AxisListType.X,
                                    op=Alu.add)
            if h == 1:
                nc.vector.tensor_copy(s2[:], s2h[:])
            else:
                nc.vector.tensor_add(s2[:], s2[:], s2h[:])

        s_tot = pool.tile([128, 1], f32)
        nc.vector.tensor_reduce(s_tot[:], ssum[:], axis=mybir.AxisListType.X,
                                op=Alu.add)
        lse = pool.tile([128, 1], f32)
        nc.scalar.activation(lse[:], s_tot[:], Act.Ln)

        ps_ls = psum.tile([128, 1], f32, tag="sm1")
        nc.tensor.matmul(ps_ls[:], onehot_sb[:], lse[:], start=True, stop=True)

        invlen = pool.tile([128, 1], f32)
        nc.vector.reciprocal(invlen[:], len_sb[:])

        # kl_slot = (s1 - s2 + lse_u*s0) * piw * invlen / U
        t0 = pool.tile([128, 1], f32)
        nc.vector.tensor_sub(t0[:], s1[:], s2[:])
        t1 = pool.tile([128, 1], f32)
        nc.vector.tensor_mul(t1[:], ps_ls[:], s0[:])
        t2 = pool.tile([128, 1], f32)
        nc.vector.tensor_add(t2[:], t0[:], t1[:])
        t3 = pool.tile([128, 1], f32)
        nc.vector.tensor_mul(t3[:], t2[:], piw_sb[:])
        t4 = pool.tile([128, 1], f32)
        nc.vector.tensor_mul(t4[:], t3[:], invlen[:])
        pair = pool.tile([128, 2], f32)
        nc.vector.tensor_scalar_mul(pair[:, 1:2], t4[:], 1.0 / U)

        # base_row = -(dot_row - lse*rsum)/B
        t5 = pool.tile([128, 1], f32)
        nc.vector.tensor_mul(t5[:], lse[:], rsum[:])
        t6 = pool.tile([128, 1], f32)
        nc.vector.tensor_sub(t6[:], dot_row[:], t5[:])
        nc.vector.tensor_scalar_mul(pair[:, 0:1], t6[:], -1.0 / B)

        ones1 = pool.tile([128, 1], f32)
        nc.vector.memset(ones1[:], 1.0)
        ps_fin = psum.tile([1, 2], f32, tag="sm2", bufs=2)
        nc.tensor.matmul(ps_fin[:], ones1[:], pair[:], start=True, stop=True)
        out_sb = pool.tile([1, 2], f32)
        nc.vector.tensor_copy(out_sb[:], ps_fin[:])
        nc.sync.dma_start(out=out_d[:], in_=out_sb[:])

    nc.compile()
    return nc


def get_program():
    if "nc" not in _prog_cache:
        _prog_cache["nc"] = _build_program()
    return _prog_cache["nc"]


def _pack_gather(users, inter_idx, lengths):
    """Ragged-pack per-group gather word indices, split at IHALF.

    Returns per half: wrapped int16 word-index array [128, NIDX//16],
    valid mask vm, parity mask par, inverse-parity mask ipar (all
    [128, NIDX]; ipar=1 at padding so the selected value stays > 0).
    """
    gidx = [np.zeros((128, NIDX // 16), np.int16) for _ in range(2)]
    vm = [np.zeros((128, NIDX), _BF16) for _ in range(2)]
    par = [np.zeros((128, NIDX), _BF16) for _ in range(2)]
    ipar = [np.ones((128, NIDX), _BF16) for _ in range(2)]
    jj = np.arange(NIDX)
    for g in range(8):
        us = users[16 * g:16 * (g + 1)]
        lists = [np.zeros(NIDX, np.int64) for _ in range(2)]
        pos = [0, 0]
        for s_loc, u in enumerate(us):
            il = inter_idx[u][:lengths[u]].astype(np.int64)
            for half, sel in enumerate((il < IHALF, il >= IHALF)):
                idx_h = il[sel] - half * IHALF
                n = len(idx_h)
                p0 = pos[half]
                assert p0 + n <= NIDX, "gather capacity exceeded"
                lists[half][p0:p0 + n] = idx_h >> 1
                rows = 16 * g + s_loc
                vm[half][rows, p0:p0 + n] = 1.0
                odd = (idx_h & 1).astype(_BF16)
                par[half][rows, p0:p0 + n] = odd
                # ipar defaults to 1 (padding-safe); overwrite real slots
                ipar[half][rows, p0:p0 + n] = 1.0 - odd.astype(np.float32)
                pos[half] += n
        for half in range(2):
            # positions are packed per group: every partition of the group
            # shares the same index list, wrapped across 16 partitions
            gidx[half][16 * g + (jj % 16), jj // 16] = lists[half][jj]
    # ipar rows for positions claimed by OTHER partitions in the group must
    # stay consistent with par: val is only consumed where vm=1, but keep
    # par+ipar <= 1 to avoid overflow concerns; nothing else needed.
    return gidx, vm, par, ipar


def make_in_maps(rating_vec, enc_w, enc_b, dec_w, dec_b, before_score, piw,
                 batch_idx, inter_idx, lengths):
    """Host-side sharding / layout prep. Index arithmetic + casts only."""
    f32 = np.float32
    rating_vec = np.asarray(rating_vec, f32)
    enc_w = np.asarray(enc_w, f32)
    enc_b = np.asarray(enc_b, f32)
    dec_w = np.asarray(dec_w, f32)
    dec_b = np.asarray(dec_b, f32)
    before_score = np.asarray(before_score, f32)
    piw = np.asarray(piw, f32)
    batch_idx = np.asarray(batch_idx)
    inter_idx = np.asarray(inter_idx)
    lengths = np.asarray(lengths)

    # shared (replicated) tensors
    xmat = np.zeros((KT * 128, 402), f32)
    xmat[:I, 0:D] = enc_w.T
    xmat[:I, D:2 * D] = dec_w
    xmat[:I, 2 * D] = dec_b
    xmat[:I, 2 * D + 1] = 1.0
    xmat[I, 0:D] = enc_b
    xmat_bf = np.ascontiguousarray(
        xmat.astype(_BF16).reshape(NG, KG, 128, 402).transpose(0, 2, 1, 3))

    dwt = dec_w.T  # [200, 20000]
    dwta = np.ascontiguousarray(dwt[:128]).astype(_BF16)
    dwtb = np.concatenate([dwt[128:D], dec_b[None, :]], axis=0).astype(_BF16)

    ident = np.eye(128, dtype=_BF16)

    in_maps = []
    for c in range(NCORES):
        r0 = BSH * c
        ratingT = np.zeros((KT * 128, BSH), f32)
        ratingT[:I] = rating_vec[r0:r0 + BSH].T
        ratingT[I] = 1.0

        users = np.nonzero((batch_idx >= r0) & (batch_idx < r0 + BSH))[0]
        nu = len(users)
        assert nu <= UCAP, f"core {c}: {nu} users > capacity {UCAP}"

        bef = np.empty((UCAP, I), _BF16)
        bef[:nu] = before_score[users]
        bef[nu:] = before_score[0]

        gidx, vm, par, ipar = _pack_gather(users, inter_idx, lengths)

        onehot_arr = np.zeros((128, UCAP), f32)
        onehot_arr[batch_idx[users] - r0, np.arange(nu)] = 1.0

        piw_arr = np.zeros((UCAP, 1), f32)
        piw_arr[:nu, 0] = piw[users]
        len_arr = np.ones((UCAP, 1), f32)
        len_arr[:nu, 0] = lengths[users].astype(f32)

        in_maps.append(dict(
            ratingT=np.ascontiguousarray(
                ratingT.astype(_BF16).reshape(NG, KG, 128, BSH)
                .transpose(0, 2, 1, 3)),
            xmat=xmat_bf,
            dwta=dwta,
            dwtb=dwtb,
            bef_0=np.ascontiguousarray(bef[:, :IHALF]),
            bef_1=np.ascontiguousarray(bef[:, IHALF:]),
            gidx_0=gidx[0], gidx_1=gidx[1],
            vm_0=vm[0], vm_1=vm[1],
            par_0=par[0], par_1=par[1],
            ipar_0=ipar[0], ipar_1=ipar[1],
            onehot=onehot_arr,
            ident=ident,
            piw_s=piw_arr,
            len_s=len_arr,
        ))
    return in_maps


def combine(outs):
    base = f32sum(o[0, 0] for o in outs)
    kl = f32sum(o[0, 1] for o in outs)
    return np.float32(base), np.float32(kl)


def f32sum(it):
    acc = np.float32(0.0)
    for v in it:
        acc = np.float32(acc + np.float32(v))
    return acc


def kernel(**inputs):
    nc = get_program()
    in_maps = make_in_maps(**inputs)
    from concourse.bass_utils import run_bass_kernel_spmd
    res = run_bass_kernel_spmd(nc, in_maps, list(range(NCORES)))
    outs = [res.results[c]["out"] for c in range(NCORES)]
    return combine(outs)


# revision 30
# speedup vs baseline: 1.0026x; 1.0026x over previous
"""Trainium2 Bass kernel for nn_CL_VAE (Multi-VAE loss + contrastive-learning KL).

Strategy (8 NeuronCores, data-parallel over batch rows + common users):
  - core c owns batch rows [128c, 128c+128) and the common users whose
    batch_idx falls in that range (padded to 128 user slots, user = SBUF
    partition).
  - Host prep is layout-only: shard/transpose/concat/cast inputs, build
    index tables + 0/1 masks from the integer index inputs.
  - Device per core:
      phase 0: gather before_score at (user, inter_idx) positions with
               gpsimd ap_gather (ragged-packed word indices + parity
               select, split by item halves), w = b_sel at valid slots,
               S0 = sum w, S1 = sum w*log(b_sel).
      phase 1: one fused matmul over K = I(+1 bias row):
               [h_pre | rating@dec_w | rating@dec_b | sum(rating)] =
               rating_aug^T.T @ [enc_w^T | dec_w | dec_b | 1].
      phase 2: decoder matmuls logits = [h|1] @ [dec_w^T; dec_b] for both
               batch rows and user rows from resident weights, high item
               half first (so its user-logit gather overlaps the low
               half); exp+accumulate for softmax denominators.
      phase 3: lse = log(sum exp), gather user logits, S2 = sum(w*glog),
               combine into the two loss partials, reduce over partitions
               with a ones-matmul.
  - Host combines the 8 per-core [1,2] partials by summation only.

log-softmax identity used (exact in real arithmetic):
  sum_i log_probs*rating = sum_i logits*rating - lse*sum_i rating
  log(p_sel) = logits_sel - lse      (since p_sel = exp(log_probs))
The +EPS terms of the reference cancel in (log_b - log_p) and are below
fp32 resolution elsewhere.

Gathers run on bf16 data but ap_gather moves 4-byte words, so indices
are word indices (il>>1) and a host-provided parity mask pair selects
the bf16 half: val = ipar*even + par*odd.
"""

import numpy as np
import ml_dtypes

# ---- hardcoded problem shapes ----
B, I, D, U, L = 1024, 20000, 200, 512, 200
NCORES = 8
BSH = B // NCORES      # 128 batch rows per core
UCAP = 128             # padded user slots per core
KTOT = I + 1           # contraction length (+1 bias/ones row)
KG = 8                 # k-tiles per DMA batch in phase 1
KT = 160               # k-tiles (zero-padded up from 157)
NG = KT // KG          # 20 phase-1 DMA groups
CH = 500               # decoder free-dim chunk
NCH = I // CH          # 40
IHALF = I // 2         # item split for gather overlap (10000)
NCHH = NCH // 2        # 20 chunks per half
NIDX = 1440            # ragged-packed gather indices per 16-part group/half

_BF16 = ml_dtypes.bfloat16

_prog_cache = {}


def _build_program():
    import concourse.bacc as bacc
    import concourse.mybir as mybir
    import concourse.tile as tile
    from contextlib import ExitStack

    f32 = mybir.dt.float32
    bf16 = mybir.dt.bfloat16
    i16 = mybir.dt.int16
    Act = mybir.ActivationFunctionType
    Alu = mybir.AluOpType

    nc = bacc.Bacc("TRN2", target_bir_lowering=False, debug=False,
                   num_devices=NCORES)

    ratingT = nc.dram_tensor("ratingT", [NG, 128, KG, BSH], bf16,
                             kind="ExternalInput")
    xmat = nc.dram_tensor("xmat", [NG, 128, KG, 402], bf16,
                          kind="ExternalInput")
    dwta_d = nc.dram_tensor("dwta", [128, I], bf16, kind="ExternalInput")
    dwtb_d = nc.dram_tensor("dwtb", [73, I], bf16, kind="ExternalInput")
    bef_d = [nc.dram_tensor(f"bef_{h}", [UCAP, IHALF], bf16,
                            kind="ExternalInput") for h in range(2)]
    gidx_d = [nc.dram_tensor(f"gidx_{h}", [128, NIDX // 16], i16,
                             kind="ExternalInput") for h in range(2)]
    vm_d = [nc.dram_tensor(f"vm_{h}", [128, NIDX], bf16,
                           kind="ExternalInput") for h in range(2)]
    par_d = [nc.dram_tensor(f"par_{h}", [128, NIDX], bf16,
                            kind="ExternalInput") for h in range(2)]
    ipar_d = [nc.dram_tensor(f"ipar_{h}", [128, NIDX], bf16,
                             kind="ExternalInput") for h in range(2)]
    onehot = nc.dram_tensor("onehot", [128, UCAP], f32, kind="ExternalInput")
    ident = nc.dram_tensor("ident", [128, 128], bf16, kind="ExternalInput")
    piw_s = nc.dram_tensor("piw_s", [UCAP, 1], f32, kind="ExternalInput")
    len_s = nc.dram_tensor("len_s", [UCAP, 1], f32, kind="ExternalInput")
    out_d = nc.dram_tensor("out", [1, 2], f32, kind="ExternalOutput")

    with ExitStack() as ctx:
        tc = ctx.enter_context(tile.TileContext(nc))
        pool = ctx.enter_context(tc.tile_pool(name="main", bufs=1))
        big = ctx.enter_context(tc.tile_pool(name="big", bufs=1))
        gpool = ctx.enter_context(tc.tile_pool(name="gat", bufs=1))
        stream = ctx.enter_context(tc.tile_pool(name="stream", bufs=3))
        psum = ctx.enter_context(tc.tile_pool(name="ps", bufs=1, space="PSUM"))
        psmm = ctx.enter_context(tc.tile_pool(name="psmm", bufs=2,
                                              space="PSUM"))

        def parity_select(gw, par, ipar, out_dtype, name):
            """val = ipar*even(gw) + par*odd(gw); returns [128, NIDX] tile."""
            wb = gw[:].bitcast(bf16).rearrange("p (j t) -> p j t", t=2)
            t_e = gpool.tile([128, NIDX], out_dtype, tag="val", bufs=2,
                             name=f"{name}_e")
            nc.vector.tensor_mul(t_e[:], ipar[:], wb[:, :, 0:1])
            t_o = gpool.tile([128, NIDX], out_dtype, tag="val", bufs=2,
                             name=f"{name}_o")
            nc.vector.tensor_mul(t_o[:], par[:], wb[:, :, 1:2])
            val = gpool.tile([128, NIDX], out_dtype, tag="val2", bufs=2,
                             name=f"{name}_v")
            nc.vector.tensor_add(val[:], t_e[:], t_o[:])
            return val

        # ------- phase 1 main matmul stream (DMA priority: first) -------
        ps1 = psmm.tile([128, 402], f32, tag="mm")
        for g in range(NG):
            rt_g = stream.tile([128, KG, BSH], bf16, tag="rt", bufs=3)
            nc.sync.dma_start(out=rt_g[:], in_=ratingT[g])
            x_g = stream.tile([128, KG, 402], bf16, tag="xk", bufs=2)
            nc.sync.dma_start(out=x_g[:], in_=xmat[g])
            for kk in range(KG):
                k = KG * g + kk
                nc.tensor.matmul(ps1[:], rt_g[:, kk, :], x_g[:, kk, :],
                                 start=(k == 0), stop=(k == KT - 1))

        # All DMA issue on the sync engine: it is otherwise idle, its
        # issue cost is low, and its DMAs spread across all 16 queues.
        def dma(out, in_):
            nc.sync.dma_start(out=out, in_=in_)

        # ------- before_score halves (second; gathers gate phase 2) -------
        bef = []
        for h in (1, 0):
            bt = big.tile([UCAP, IHALF], bf16, tag=f"b{h}", name=f"bef{h}")
            for s in range(0, IHALF, 2500):
                dma(bt[:, s:s + 2500], bef_d[h][:, s:s + 2500])
            bef.append(bt)
        bef = bef[::-1]  # index by half again
        gidx, vm, par, ipar = [], [], [], []
        for h in range(2):
            gt = pool.tile([128, NIDX // 16], i16, name=f"gidx{h}")
            dma(gt[:], gidx_d[h][:])
            gidx.append(gt)

        # ------- resident decoder weights (third; high half first) -------
        dwta = pool.tile([128, I], bf16)
        dwtb = pool.tile([73, I], bf16)
        for s in range(IHALF, I, 2500):
            dma(dwta[:, s:s + 2500], dwta_d[:, s:s + 2500])
            dma(dwtb[:, s:s + 2500], dwtb_d[:, s:s + 2500])
        for s in range(0, IHALF, 2500):
            dma(dwta[:, s:s + 2500], dwta_d[:, s:s + 2500])
            dma(dwtb[:, s:s + 2500], dwtb_d[:, s:s + 2500])

        # ------- gather tables + small constants (fourth) -------
        for h in range(2):
            vt = pool.tile([128, NIDX], bf16, name=f"vm{h}")
            dma(vt[:], vm_d[h][:])
            vm.append(vt)
            pt = pool.tile([128, NIDX], bf16, name=f"par{h}")
            dma(pt[:], par_d[h][:])
            par.append(pt)
            it = pool.tile([128, NIDX], bf16, name=f"ipar{h}")
            dma(it[:], ipar_d[h][:])
            ipar.append(it)
        onehot_sb = pool.tile([128, UCAP], f32)
        dma(onehot_sb[:], onehot[:])
        ident_sb = pool.tile([128, 128], bf16)
        dma(ident_sb[:], ident[:])
        piw_sb = pool.tile([UCAP, 1], f32)
        dma(piw_sb[:], piw_s[:])
        len_sb = pool.tile([UCAP, 1], f32)
        dma(len_sb[:], len_s[:])

        # ------- phase 0: before_score gathers (idle gpsimd, early) -------
        s0 = pool.tile([128, 1], f32)
        s1 = pool.tile([128, 1], f32)
        w_t = [None, None]
        for h in (1, 0):
            gw = gpool.tile([128, NIDX], f32, tag="tmp", bufs=2,
                            name=f"bw{h}")
            nc.gpsimd.ap_gather(gw[:], bef[h][:].bitcast(f32), gidx[h][:],
                                channels=128, num_elems=IHALF // 2, d=1,
                                num_idxs=NIDX)
            val = parity_select(gw, par[h], ipar[h], bf16, f"bv{h}")
            logb = gpool.tile([128, NIDX], f32, tag="tmp", bufs=2,
                              name=f"logb{h}")
            nc.scalar.activation(logb[:], val[:], Act.Ln)
            wt = pool.tile([128, NIDX], f32, name=f"w{h}")
            nc.vector.tensor_mul(wt[:], vm[h][:], val[:])
            w_t[h] = wt
            s0h = pool.tile([128, 1], f32, tag="s0h", bufs=2)
            nc.vector.tensor_reduce(s0h[:], wt[:], axis=mybir.AxisListType.X,
                                    op=Alu.add)
            scr_a = gpool.tile([128, NIDX], f32, tag="tmp", bufs=2,
                               name=f"wl{h}")
            nc.vector.tensor_mul(scr_a[:], wt[:], logb[:])
            s1h = pool.tile([128, 1], f32, tag="s1h", bufs=2)
            nc.vector.tensor_reduce(s1h[:], scr_a[:],
                                    axis=mybir.AxisListType.X, op=Alu.add)
            if h == 1:
                nc.vector.tensor_copy(s0[:], s0h[:])
                nc.vector.tensor_copy(s1[:], s1h[:])
            else:
                nc.vector.tensor_add(s0[:], s0[:], s0h[:])
                nc.vector.tensor_add(s1[:], s1[:], s1h[:])

        # ------- phase 1 epilogue -------
        h_f = pool.tile([128, D], f32)
        nc.scalar.activation(h_f[:], ps1[:, 0:D], Act.Tanh)
        h_bf = pool.tile([128, D + 1], bf16)
        nc.vector.tensor_copy(h_bf[:, 0:D], h_f[:])
        nc.vector.memset(h_bf[:, D:D + 1], 1.0)
        scr200 = pool.tile([128, D], f32)
        nc.vector.tensor_mul(scr200[:], h_f[:], ps1[:, D:2 * D])
        dot_p = pool.tile([128, 1], f32)
        nc.vector.tensor_reduce(dot_p[:], scr200[:], axis=mybir.AxisListType.X,
                                op=Alu.add)
        dot_row = pool.tile([128, 1], f32)
        nc.vector.tensor_add(dot_row[:], dot_p[:], ps1[:, 2 * D:2 * D + 1])
        rsum = pool.tile([128, 1], f32)
        nc.vector.tensor_copy(rsum[:], ps1[:, 2 * D + 1:2 * D + 2])

        ps_hu = psum.tile([128, D], f32, tag="sm1")
        nc.tensor.matmul(ps_hu[:], onehot_sb[:], h_f[:], start=True, stop=True)
        hu_bf = pool.tile([128, D + 1], bf16)
        nc.vector.tensor_copy(hu_bf[:, 0:D], ps_hu[:])
        nc.vector.memset(hu_bf[:, D:D + 1], 1.0)

        # transposes: h^T and h_u^T, each augmented with a trailing ones row
        hta = pool.tile([128, BSH], bf16)
        htb = pool.tile([73, BSH], bf16)
        huta = pool.tile([128, UCAP], bf16)
        hutb = pool.tile([73, UCAP], bf16)
        for src, dsta, dstb in ((h_bf, hta, htb), (hu_bf, huta, hutb)):
            ps_t1 = psum.tile([128, 128], bf16, tag="sm2", bufs=2)
            nc.tensor.transpose(ps_t1[:], src[:, 0:128], ident_sb[:])
            nc.vector.tensor_copy(dsta[:], ps_t1[:])
            ps_t2 = psum.tile([73, 128], bf16, tag="sm2", bufs=2)
            nc.tensor.transpose(ps_t2[:], src[:, 128:D + 1], ident_sb[:])
            nc.vector.tensor_copy(dstb[:], ps_t2[:])

        # ------- phase 2: decoder stream, high item half first -------
        ulog = [None, None]
        ulog[1] = big.tile([UCAP, IHALF], bf16, tag="b1", name="ulog1")
        ulog[0] = big.tile([UCAP, IHALF], bf16, tag="b0", name="ulog0")
        ssum = pool.tile([128, NCH], f32)
        est = stream.tile([128, CH], f32, tag="est", bufs=1)
        for ci in range(NCH):
            c = ci + NCHH if ci < NCHH else ci - NCHH  # hi half first
            das = dwta[:, CH * c:CH * (c + 1)]
            dbs = dwtb[:, CH * c:CH * (c + 1)]
            pm = psmm.tile([128, CH], f32, tag="mm")
            nc.tensor.matmul(pm[:], hta[:], das, start=True, stop=False)
            nc.tensor.matmul(pm[:], htb[:], dbs, start=False, stop=True)
            pu = psmm.tile([128, CH], f32, tag="pu")
            nc.tensor.matmul(pu[:], huta[:], das, start=True, stop=False)
            nc.tensor.matmul(pu[:], hutb[:], dbs, start=False, stop=True)
            nc.scalar.activation(est[:], pm[:], Act.Exp,
                                 accum_out=ssum[:, c:c + 1])
            half = 1 if c >= NCHH else 0
            c0 = CH * (c - NCHH) if c >= NCHH else CH * c
            nc.vector.tensor_copy(ulog[half][:, c0:c0 + CH], pu[:])

        # ------- phase 3: lse, user-logit gathers, combine -------
        s2 = pool.tile([128, 1], f32)
        for h in (1, 0):
            gw = gpool.tile([128, NIDX], f32, tag="tmp", bufs=2,
                            name=f"uw{h}")
            nc.gpsimd.ap_gather(gw[:], ulog[h][:].bitcast(f32), gidx[h][:],
                                channels=128, num_elems=IHALF // 2, d=1,
                                num_idxs=NIDX)
            valg = parity_select(gw, par[h], ipar[h], f32, f"uv{h}")
            scr = gpool.tile([128, NIDX], f32, tag="tmp", bufs=2,
                             name=f"ws{h}")
            nc.vector.tensor_mul(scr[:], w_t[h][:], valg[:])
            s2h = pool.tile([128, 1], f32, tag="s2h", bufs=2)
            nc.vector.tensor_reduce(s2h[:], scr[:], axis=mybir.AxisListType.X,
                                    op=Alu.add)
            if h == 1:
                nc.vector.tensor_copy(s2[:], s2h[:])
            else:
                nc.vector.tensor_add(s2[:], s2[:], s2h[:])

        s_tot = pool.tile([128, 1], f32)
        nc.vector.tensor_reduce(s_tot[:], ssum[:], axis=mybir.AxisListType.X,
                                op=Alu.add)
        lse = pool.tile([128, 1], f32)
        nc.scalar.activation(lse[:], s_tot[:], Act.Ln)

        ps_ls = psum.tile([128, 1], f32, tag="sm1")
        nc.tensor.matmul(ps_ls[:], onehot_sb[:], lse[:], start=True, stop=True)

        invlen = pool.tile([128, 1], f32)
        nc.vector.reciprocal(invlen[:], len_sb[:])

        # kl_slot = (s1 - s2 + lse_u*s0) * piw * invlen / U
        t0 = pool.tile([128, 1], f32)
        nc.vector.tensor_sub(t0[:], s1[:], s2[:])
        t1 = pool.tile([128, 1], f32)
        nc.vector.tensor_mul(t1[:], ps_ls[:], s0[:])
        t2 = pool.tile([128, 1], f32)
        nc.vector.tensor_add(t2[:], t0[:], t1[:])
        t3 = pool.tile([128, 1], f32)
        nc.vector.tensor_mul(t3[:], t2[:], piw_sb[:])
        t4 = pool.tile([128, 1], f32)
        nc.vector.tensor_mul(t4[:], t3[:], invlen[:])
        pair = pool.tile([128, 2], f32)
        nc.vector.tensor_scalar_mul(pair[:, 1:2], t4[:], 1.0 / U)

        # base_row = -(dot_row - lse*rsum)/B
        t5 = pool.tile([128, 1], f32)
        nc.vector.tensor_mul(t5[:], lse[:], rsum[:])
        t6 = pool.tile([128, 1], f32)
        nc.vector.tensor_sub(t6[:], dot_row[:], t5[:])
        nc.vector.tensor_scalar_mul(pair[:, 0:1], t6[:], -1.0 / B)

        ones1 = pool.tile([128, 1], f32)
        nc.vector.memset(ones1[:], 1.0)
        ps_fin = psum.tile([1, 2], f32, tag="sm2", bufs=2)
        nc.tensor.matmul(ps_fin[:], ones1[:], pair[:], start=True, stop=True)
        out_sb = pool.tile([1, 2], f32)
        nc.vector.tensor_copy(out_sb[:], ps_fin[:])
        nc.sync.dma_start(out=out_d[:], in_=out_sb[:])

    nc.compile()
    return nc


def get_program():
    if "nc" not in _prog_cache:
        _prog_cache["nc"] = _build_program()
    return _prog_cache["nc"]


def _pack_gather(users, inter_idx, lengths):
    """Ragged-pack per-group gather word indices, split at IHALF.

    Returns per half: wrapped int16 word-index array [128, NIDX//16],
    valid mask vm, parity mask par, inverse-parity mask ipar (all
    [128, NIDX]; ipar=1 at padding so the selected value stays > 0).
    """
    gidx = [np.zeros((128, NIDX // 16), np.int16) for _ in range(2)]
    vm = [np.zeros((128, NIDX), _BF16) for _ in range(2)]
    par = [np.zeros((128, NIDX), _BF16) for _ in range(2)]
    ipar = [np.ones((128, NIDX), _BF16) for _ in range(2)]
    jj = np.arange(NIDX)
    for g in range(8):
        us = users[16 * g:16 * (g + 1)]
        lists = [np.zeros(NIDX, np.int64) for _ in range(2)]
        pos = [0, 0]
        for s_loc, u in enumerate(us):
            il = inter_idx[u][:lengths[u]].astype(np.int64)
            for half, sel in enumerate((il < IHALF, il >= IHALF)):
                idx_h = il[sel] - half * IHALF
                n = len(idx_h)
                p0 = pos[half]
                assert p0 + n <= NIDX, "gather capacity exceeded"
                lists[half][p0:p0 + n] = idx_h >> 1
                rows = 16 * g + s_loc
                vm[half][rows, p0:p0 + n] = 1.0
                odd = (idx_h & 1).astype(_BF16)
                par[half][rows, p0:p0 + n] = odd
                # ipar defaults to 1 (padding-safe); overwrite real slots
                ipar[half][rows, p0:p0 + n] = 1.0 - odd.astype(np.float32)
                pos[half] += n
        for half in range(2):
            # positions are packed per group: every partition of the group
            # shares the same index list, wrapped across 16 partitions
            gidx[half][16 * g + (jj % 16), jj // 16] = lists[half][jj]
    # ipar rows for positions claimed by OTHER partitions in the group must
    # stay consistent with par: val is only consumed where vm=1, but keep
    # par+ipar <= 1 to avoid overflow concerns; nothing else needed.
    return gidx, vm, par, ipar


def make_in_maps(rating_vec, enc_w, enc_b, dec_w, dec_b, before_score, piw,
                 batch_idx, inter_idx, lengths):
    """Host-side sharding / layout prep. Index arithmetic + casts only."""
    f32 = np.float32
    rating_vec = np.asarray(rating_vec, f32)
    enc_w = np.asarray(enc_w, f32)
    enc_b = np.asarray(enc_b, f32)
    dec_w = np.asarray(dec_w, f32)
    dec_b = np.asarray(dec_b, f32)
    before_score = np.asarray(before_score, f32)
    piw = np.asarray(piw, f32)
    batch_idx = np.asarray(batch_idx)
    inter_idx = np.asarray(inter_idx)
    lengths = np.asarray(lengths)

    # shared (replicated) tensors
    xmat = np.zeros((KT * 128, 402), f32)
    xmat[:I, 0:D] = enc_w.T
    xmat[:I, D:2 * D] = dec_w
    xmat[:I, 2 * D] = dec_b
    xmat[:I, 2 * D + 1] = 1.0
    xmat[I, 0:D] = enc_b
    xmat_bf = np.ascontiguousarray(
        xmat.astype(_BF16).reshape(NG, KG, 128, 402).transpose(0, 2, 1, 3))

    dwt = dec_w.T  # [200, 20000]
    dwta = np.ascontiguousarray(dwt[:128]).astype(_BF16)
    dwtb = np.concatenate([dwt[128:D], dec_b[None, :]], axis=0).astype(_BF16)

    ident = np.eye(128, dtype=_BF16)

    in_maps = []
    for c in range(NCORES):
        r0 = BSH * c
        ratingT = np.zeros((KT * 128, BSH), f32)
        ratingT[:I] = rating_vec[r0:r0 + BSH].T
        ratingT[I] = 1.0

        users = np.nonzero((batch_idx >= r0) & (batch_idx < r0 + BSH))[0]
        nu = len(users)
        assert nu <= UCAP, f"core {c}: {nu} users > capacity {UCAP}"

        bef = np.empty((UCAP, I), _BF16)
        bef[:nu] = before_score[users]
        bef[nu:] = before_score[0]

        gidx, vm, par, ipar = _pack_gather(users, inter_idx, lengths)

        onehot_arr = np.zeros((128, UCAP), f32)
        onehot_arr[batch_idx[users] - r0, np.arange(nu)] = 1.0

        piw_arr = np.zeros((UCAP, 1), f32)
        piw_arr[:nu, 0] = piw[users]
        len_arr = np.ones((UCAP, 1), f32)
        len_arr[:nu, 0] = lengths[users].astype(f32)

        in_maps.append(dict(
            ratingT=np.ascontiguousarray(
                ratingT.astype(_BF16).reshape(NG, KG, 128, BSH)
                .transpose(0, 2, 1, 3)),
            xmat=xmat_bf,
            dwta=dwta,
            dwtb=dwtb,
            bef_0=np.ascontiguousarray(bef[:, :IHALF]),
            bef_1=np.ascontiguousarray(bef[:, IHALF:]),
            gidx_0=gidx[0], gidx_1=gidx[1],
            vm_0=vm[0], vm_1=vm[1],
            par_0=par[0], par_1=par[1],
            ipar_0=ipar[0], ipar_1=ipar[1],
            onehot=onehot_arr,
            ident=ident,
            piw_s=piw_arr,
            len_s=len_arr,
        ))
    return in_maps


def combine(outs):
    base = f32sum(o[0, 0] for o in outs)
    kl = f32sum(o[0, 1] for o in outs)
    return np.float32(base), np.float32(kl)


def f32sum(it):
    acc = np.float32(0.0)
    for v in it:
        acc = np.float32(acc + np.float32(v))
    return acc


def kernel(**inputs):
    nc = get_program()
    in_maps = make_in_maps(**inputs)
    from concourse.bass_utils import run_bass_kernel_spmd
    res = run_bass_kernel_spmd(nc, in_maps, list(range(NCORES)))
    outs = [res.results[c]["out"] for c in range(NCORES)]
    return combine(outs)


# revision 35
# speedup vs baseline: 1.1327x; 1.1298x over previous
"""Trainium2 Bass kernel for nn_CL_VAE (Multi-VAE loss + contrastive-learning KL).

Strategy (8 NeuronCores, data-parallel over batch rows + common users):
  - core c owns batch rows [128c, 128c+128) and the common users whose
    batch_idx falls in that range (padded to 128 user slots, user = SBUF
    partition).
  - Host prep is layout-only: shard/transpose/concat/cast inputs, build
    index tables + 0/1 masks from the integer index inputs.
  - Device per core:
      phase 0: gather before_score at (user, inter_idx) positions with
               gpsimd ap_gather (ragged-packed word indices + parity
               select, split by item halves), w = b_sel at valid slots,
               S0 = sum w, S1 = sum w*log(b_sel).
      phase 1: one fused matmul over K = I(+1 bias row):
               [h_pre | rating@dec_w | rating@dec_b | sum(rating)] =
               rating_aug^T.T @ [enc_w^T | dec_w | dec_b | 1].
      phase 2: decoder matmuls logits = [h|1] @ [dec_w^T; dec_b] for both
               batch rows and user rows from resident weights, high item
               half first (so its user-logit gather overlaps the low
               half); exp+accumulate for softmax denominators.
      phase 3: lse = log(sum exp), gather user logits, S2 = sum(w*glog),
               combine into the two loss partials, reduce over partitions
               with a ones-matmul.
  - Host combines the 8 per-core [1,2] partials by summation only.

log-softmax identity used (exact in real arithmetic):
  sum_i log_probs*rating = sum_i logits*rating - lse*sum_i rating
  log(p_sel) = logits_sel - lse      (since p_sel = exp(log_probs))
The +EPS terms of the reference cancel in (log_b - log_p) and are below
fp32 resolution elsewhere.

Gathers run on bf16 data but ap_gather moves 4-byte words, so indices
are word indices (il>>1) and a host-provided parity mask pair selects
the bf16 half: val = ipar*even + par*odd.
"""

import numpy as np
import ml_dtypes

# ---- hardcoded problem shapes ----
B, I, D, U, L = 1024, 20000, 200, 512, 200
NCORES = 8
BSH = B // NCORES      # 128 batch rows per core
UCAP = 128             # padded user slots per core
KTOT = I + 1           # contraction length (+1 bias/ones row)
KG = 8                 # k-tiles per DMA batch in phase 1
KT = 160               # k-tiles (zero-padded up from 157)
NG = KT // KG          # 20 phase-1 DMA groups
CH = 500               # decoder free-dim chunk
NCH = I // CH          # 40
IHALF = I // 2         # item split for gather overlap (10000)
NCHH = NCH // 2        # 20 chunks per half
NIDX = 1440            # ragged-packed gather indices per 16-part group/half

# packed small-constant blob layout: per-partition byte (offset, size)
_B2 = NIDX * 2
BLOB_LAYOUT = {
    "vm0": (0, _B2), "vm1": (_B2, _B2),
    "par0": (2 * _B2, _B2), "par1": (3 * _B2, _B2),
    "ipar0": (4 * _B2, _B2), "ipar1": (5 * _B2, _B2),
    "gidx0": (6 * _B2, NIDX // 8), "gidx1": (6 * _B2 + NIDX // 8, NIDX // 8),
    "onehot": (6 * _B2 + NIDX // 4, 512),
    "ident": (6 * _B2 + NIDX // 4 + 512, 256),
    "piw": (6 * _B2 + NIDX // 4 + 768, 4),
    "len": (6 * _B2 + NIDX // 4 + 772, 4),
}
BLOB_BYTES = 6 * _B2 + NIDX // 4 + 776

_BF16 = ml_dtypes.bfloat16

_prog_cache = {}


def _build_program():
    import concourse.bacc as bacc
    import concourse.mybir as mybir
    import concourse.tile as tile
    from contextlib import ExitStack

    f32 = mybir.dt.float32
    bf16 = mybir.dt.bfloat16
    i16 = mybir.dt.int16
    Act = mybir.ActivationFunctionType
    Alu = mybir.AluOpType

    nc = bacc.Bacc("TRN2", target_bir_lowering=False, debug=False,
                   num_devices=NCORES)

    ratingT = nc.dram_tensor("ratingT", [NG, 128, KG, BSH], bf16,
                             kind="ExternalInput")
    xmat = nc.dram_tensor("xmat", [NG, 128, KG, 402], bf16,
                          kind="ExternalInput")
    dwta_d = nc.dram_tensor("dwta", [128, I], bf16, kind="ExternalInput")
    dwtb_d = nc.dram_tensor("dwtb", [73, I], bf16, kind="ExternalInput")
    bef_d = [nc.dram_tensor(f"bef_{h}", [UCAP, IHALF], bf16,
                            kind="ExternalInput") for h in range(2)]
    u8 = mybir.dt.uint8
    blob_d = nc.dram_tensor("blob", [128, BLOB_BYTES], u8,
                            kind="ExternalInput")
    out_d = nc.dram_tensor("out", [1, 2], f32, kind="ExternalOutput")

    with ExitStack() as ctx:
        tc = ctx.enter_context(tile.TileContext(nc))
        pool = ctx.enter_context(tc.tile_pool(name="main", bufs=1))
        big = ctx.enter_context(tc.tile_pool(name="big", bufs=1))
        gpool = ctx.enter_context(tc.tile_pool(name="gat", bufs=1))
        stream = ctx.enter_context(tc.tile_pool(name="stream", bufs=3))
        psum = ctx.enter_context(tc.tile_pool(name="ps", bufs=1, space="PSUM"))
        psmm = ctx.enter_context(tc.tile_pool(name="psmm", bufs=2,
                                              space="PSUM"))

        def parity_select(gw, par_ap, ipar_ap, out_dtype, name):
            """val = ipar*even(gw) + par*odd(gw); returns [128, NIDX] tile."""
            wb = gw[:].bitcast(bf16).rearrange("p (j t) -> p j t", t=2)
            t_e = gpool.tile([128, NIDX], out_dtype, tag="val", bufs=2,
                             name=f"{name}_e")
            nc.vector.tensor_mul(t_e[:], ipar_ap, wb[:, :, 0:1])
            t_o = gpool.tile([128, NIDX], out_dtype, tag="val", bufs=2,
                             name=f"{name}_o")
            nc.vector.tensor_mul(t_o[:], par_ap, wb[:, :, 1:2])
            val = gpool.tile([128, NIDX], out_dtype, tag="val2", bufs=2,
                             name=f"{name}_v")
            nc.vector.tensor_add(val[:], t_e[:], t_o[:])
            return val

        # ------- phase 1 main matmul stream (DMA priority: first) -------
        ps1 = psmm.tile([128, 402], f32, tag="mm")
        for g in range(NG):
            rt_g = stream.tile([128, KG, BSH], bf16, tag="rt", bufs=3)
            nc.sync.dma_start(out=rt_g[:], in_=ratingT[g])
            x_g = stream.tile([128, KG, 402], bf16, tag="xk", bufs=2)
            nc.sync.dma_start(out=x_g[:], in_=xmat[g])
            for kk in range(KG):
                k = KG * g + kk
                nc.tensor.matmul(ps1[:], rt_g[:, kk, :], x_g[:, kk, :],
                                 start=(k == 0), stop=(k == KT - 1))

        # ------- scalar-queue traffic: bef, dwta, blob (sync has xmat) ----
        bef = [None, None]
        for h in (1, 0):
            bt = big.tile([UCAP, IHALF], bf16, tag=f"b{h}", name=f"bef{h}")
            nc.scalar.dma_start(out=bt[:], in_=bef_d[h][:])
            bef[h] = bt
        blob_sb = pool.tile([128, BLOB_BYTES], mybir.dt.uint8)
        nc.scalar.dma_start(out=blob_sb[:], in_=blob_d[:])
        dwta = pool.tile([128, I], bf16)
        nc.scalar.dma_start(out=dwta[:, IHALF:I], in_=dwta_d[:, IHALF:I])
        nc.scalar.dma_start(out=dwta[:, 0:IHALF], in_=dwta_d[:, 0:IHALF])
        dwtb = pool.tile([73, I], bf16)
        nc.sync.dma_start(out=dwtb[:, IHALF:I], in_=dwtb_d[:, IHALF:I])
        nc.sync.dma_start(out=dwtb[:, 0:IHALF], in_=dwtb_d[:, 0:IHALF])

        def _bview(name, dtype):
            off, sz = BLOB_LAYOUT[name]
            return blob_sb[:, off:off + sz].bitcast(dtype)

        vm = [_bview("vm0", bf16), _bview("vm1", bf16)]
        par = [_bview("par0", bf16), _bview("par1", bf16)]
        ipar = [_bview("ipar0", bf16), _bview("ipar1", bf16)]
        gidx = [_bview("gidx0", i16), _bview("gidx1", i16)]
        onehot_sb = _bview("onehot", f32)
        ident_sb = _bview("ident", bf16)
        piw_sb = _bview("piw", f32)
        len_sb = _bview("len", f32)

        # ------- phase 0: before_score gathers (idle gpsimd, early) -------
        s0 = pool.tile([128, 1], f32)
        s1 = pool.tile([128, 1], f32)
        w_t = [None, None]
        for h in (1, 0):
            gw = gpool.tile([128, NIDX], f32, tag="tmp", bufs=2,
                            name=f"bw{h}")
            nc.gpsimd.ap_gather(gw[:], bef[h][:].bitcast(f32), gidx[h],
                                channels=128, num_elems=IHALF // 2, d=1,
                                num_idxs=NIDX)
            val = parity_select(gw, par[h], ipar[h], bf16, f"bv{h}")
            logb = gpool.tile([128, NIDX], f32, tag="tmp", bufs=2,
                              name=f"logb{h}")
            nc.scalar.activation(logb[:], val[:], Act.Ln)
            wt = pool.tile([128, NIDX], f32, name=f"w{h}")
            nc.vector.tensor_mul(wt[:], vm[h], val[:])
            w_t[h] = wt
            s0h = pool.tile([128, 1], f32, tag="s0h", bufs=2)
            nc.vector.tensor_reduce(s0h[:], wt[:], axis=mybir.AxisListType.X,
                                    op=Alu.add)
            scr_a = gpool.tile([128, NIDX], f32, tag="tmp", bufs=2,
                               name=f"wl{h}")
            nc.vector.tensor_mul(scr_a[:], wt[:], logb[:])
            s1h = pool.tile([128, 1], f32, tag="s1h", bufs=2)
            nc.vector.tensor_reduce(s1h[:], scr_a[:],
                                    axis=mybir.AxisListType.X, op=Alu.add)
            if h == 1:
                nc.vector.tensor_copy(s0[:], s0h[:])
                nc.vector.tensor_copy(s1[:], s1h[:])
            else:
                nc.vector.tensor_add(s0[:], s0[:], s0h[:])
                nc.vector.tensor_add(s1[:], s1[:], s1h[:])

        # ------- phase 1 epilogue -------
        h_f = pool.tile([128, D], f32)
        nc.scalar.activation(h_f[:], ps1[:, 0:D], Act.Tanh)
        h_bf = pool.tile([128, D + 1], bf16)
        nc.vector.tensor_copy(h_bf[:, 0:D], h_f[:])
        nc.vector.memset(h_bf[:, D:D + 1], 1.0)
        scr200 = pool.tile([128, D], f32)
        nc.vector.tensor_mul(scr200[:], h_f[:], ps1[:, D:2 * D])
        dot_p = pool.tile([128, 1], f32)
        nc.vector.tensor_reduce(dot_p[:], scr200[:], axis=mybir.AxisListType.X,
                                op=Alu.add)
        dot_row = pool.tile([128, 1], f32)
        nc.vector.tensor_add(dot_row[:], dot_p[:], ps1[:, 2 * D:2 * D + 1])
        rsum = pool.tile([128, 1], f32)
        nc.vector.tensor_copy(rsum[:], ps1[:, 2 * D + 1:2 * D + 2])

        ps_hu = psum.tile([128, D], f32, tag="sm1")
        nc.tensor.matmul(ps_hu[:], onehot_sb, h_f[:], start=True, stop=True)
        hu_bf = pool.tile([128, D + 1], bf16)
        nc.vector.tensor_copy(hu_bf[:, 0:D], ps_hu[:])
        nc.vector.memset(hu_bf[:, D:D + 1], 1.0)

        # transposes: h^T and h_u^T, each augmented with a trailing ones row
        hta = pool.tile([128, BSH], bf16)
        htb = pool.tile([73, BSH], bf16)
        huta = pool.tile([128, UCAP], bf16)
        hutb = pool.tile([73, UCAP], bf16)
        for src, dsta, dstb in ((h_bf, hta, htb), (hu_bf, huta, hutb)):
            ps_t1 = psum.tile([128, 128], bf16, tag="sm2", bufs=2)
            nc.tensor.transpose(ps_t1[:], src[:, 0:128], ident_sb)
            nc.vector.tensor_copy(dsta[:], ps_t1[:])
            ps_t2 = psum.tile([73, 128], bf16, tag="sm2", bufs=2)
            nc.tensor.transpose(ps_t2[:], src[:, 128:D + 1], ident_sb)
            nc.vector.tensor_copy(dstb[:], ps_t2[:])

        # ------- phase 2: decoder stream, high item half first -------
        ulog = [None, None]
        ulog[1] = big.tile([UCAP, IHALF], bf16, tag="b1", name="ulog1")
        ulog[0] = big.tile([UCAP, IHALF], bf16, tag="b0", name="ulog0")
        ssum = pool.tile([128, NCH], f32)
        est = stream.tile([128, CH], f32, tag="est", bufs=1)
        for ci in range(NCH):
            c = ci + NCHH if ci < NCHH else ci - NCHH  # hi half first
            das = dwta[:, CH * c:CH * (c + 1)]
            dbs = dwtb[:, CH * c:CH * (c + 1)]
            pm = psmm.tile([128, CH], f32, tag="mm")
            nc.tensor.matmul(pm[:], hta[:], das, start=True, stop=False)
            nc.tensor.matmul(pm[:], htb[:], dbs, start=False, stop=True)
            pu = psmm.tile([128, CH], f32, tag="pu")
            nc.tensor.matmul(pu[:], huta[:], das, start=True, stop=False)
            nc.tensor.matmul(pu[:], hutb[:], dbs, start=False, stop=True)
            nc.scalar.activation(est[:], pm[:], Act.Exp,
                                 accum_out=ssum[:, c:c + 1])
            half = 1 if c >= NCHH else 0
            c0 = CH * (c - NCHH) if c >= NCHH else CH * c
            nc.vector.tensor_copy(ulog[half][:, c0:c0 + CH], pu[:])

        # ------- phase 3: lse, user-logit gathers, combine -------
        s2 = pool.tile([128, 1], f32)
        for h in (1, 0):
            gw = gpool.tile([128, NIDX], f32, tag="tmp", bufs=2,
                            name=f"uw{h}")
            nc.gpsimd.ap_gather(gw[:], ulog[h][:].bitcast(f32), gidx[h],
                                channels=128, num_elems=IHALF // 2, d=1,
                                num_idxs=NIDX)
            valg = parity_select(gw, par[h], ipar[h], f32, f"uv{h}")
            scr = gpool.tile([128, NIDX], f32, tag="tmp", bufs=2,
                             name=f"ws{h}")
            nc.vector.tensor_mul(scr[:], w_t[h][:], valg[:])
            s2h = pool.tile([128, 1], f32, tag="s2h", bufs=2)
            nc.vector.tensor_reduce(s2h[:], scr[:], axis=mybir.AxisListType.X,
                                    op=Alu.add)
            if h == 1:
                nc.vector.tensor_copy(s2[:], s2h[:])
            else:
                nc.vector.tensor_add(s2[:], s2[:], s2h[:])

        s_tot = pool.tile([128, 1], f32)
        nc.vector.tensor_reduce(s_tot[:], ssum[:], axis=mybir.AxisListType.X,
                                op=Alu.add)
        lse = pool.tile([128, 1], f32)
        nc.scalar.activation(lse[:], s_tot[:], Act.Ln)

        ps_ls = psum.tile([128, 1], f32, tag="sm1")
        nc.tensor.matmul(ps_ls[:], onehot_sb, lse[:], start=True, stop=True)

        invlen = pool.tile([128, 1], f32)
        nc.vector.reciprocal(invlen[:], len_sb)

        # kl_slot = (s1 - s2 + lse_u*s0) * piw * invlen / U
        t0 = pool.tile([128, 1], f32)
        nc.vector.tensor_sub(t0[:], s1[:], s2[:])
        t1 = pool.tile([128, 1], f32)
        nc.vector.tensor_mul(t1[:], ps_ls[:], s0[:])
        t2 = pool.tile([128, 1], f32)
        nc.vector.tensor_add(t2[:], t0[:], t1[:])
        t3 = pool.tile([128, 1], f32)
        nc.vector.tensor_mul(t3[:], t2[:], piw_sb)
        t4 = pool.tile([128, 1], f32)
        nc.vector.tensor_mul(t4[:], t3[:], invlen[:])
        pair = pool.tile([128, 2], f32)
        nc.vector.tensor_scalar_mul(pair[:, 1:2], t4[:], 1.0 / U)

        # base_row = -(dot_row - lse*rsum)/B
        t5 = pool.tile([128, 1], f32)
        nc.vector.tensor_mul(t5[:], lse[:], rsum[:])
        t6 = pool.tile([128, 1], f32)
        nc.vector.tensor_sub(t6[:], dot_row[:], t5[:])
        nc.vector.tensor_scalar_mul(pair[:, 0:1], t6[:], -1.0 / B)

        ones1 = pool.tile([128, 1], f32)
        nc.vector.memset(ones1[:], 1.0)
        ps_fin = psum.tile([1, 2], f32, tag="sm2", bufs=2)
        nc.tensor.matmul(ps_fin[:], ones1[:], pair[:], start=True, stop=True)
        out_sb = pool.tile([1, 2], f32)
        nc.vector.tensor_copy(out_sb[:], ps_fin[:])
        nc.sync.dma_start(out=out_d[:], in_=out_sb[:])

    nc.compile()
    return nc


def get_program():
    if "nc" not in _prog_cache:
        _prog_cache["nc"] = _build_program()
    return _prog_cache["nc"]


def _pack_gather(users, inter_idx, lengths):
    """Ragged-pack per-group gather word indices, split at IHALF.

    Returns per half: wrapped int16 word-index array [128, NIDX//16],
    valid mask vm, parity mask par, inverse-parity mask ipar (all
    [128, NIDX]; ipar=1 at padding so the selected value stays > 0).
    """
    gidx = [np.zeros((128, NIDX // 16), np.int16) for _ in range(2)]
    vm = [np.zeros((128, NIDX), _BF16) for _ in range(2)]
    par = [np.zeros((128, NIDX), _BF16) for _ in range(2)]
    ipar = [np.ones((128, NIDX), _BF16) for _ in range(2)]
    jj = np.arange(NIDX)
    for g in range(8):
        us = users[16 * g:16 * (g + 1)]
        lists = [np.zeros(NIDX, np.int64) for _ in range(2)]
        pos = [0, 0]
        for s_loc, u in enumerate(us):
            il = inter_idx[u][:lengths[u]].astype(np.int64)
            for half, sel in enumerate((il < IHALF, il >= IHALF)):
                idx_h = il[sel] - half * IHALF
                n = len(idx_h)
                p0 = pos[half]
                assert p0 + n <= NIDX, "gather capacity exceeded"
                lists[half][p0:p0 + n] = idx_h >> 1
                rows = 16 * g + s_loc
                vm[half][rows, p0:p0 + n] = 1.0
                odd = (idx_h & 1).astype(_BF16)
                par[half][rows, p0:p0 + n] = odd
                # ipar defaults to 1 (padding-safe); overwrite real slots
                ipar[half][rows, p0:p0 + n] = 1.0 - odd.astype(np.float32)
                pos[half] += n
        for half in range(2):
            # positions are packed per group: every partition of the group
            # shares the same index list, wrapped across 16 partitions
            gidx[half][16 * g + (jj % 16), jj // 16] = lists[half][jj]
    # ipar rows for positions claimed by OTHER partitions in the group must
    # stay consistent with par: val is only consumed where vm=1, but keep
    # par+ipar <= 1 to avoid overflow concerns; nothing else needed.
    return gidx, vm, par, ipar


def make_in_maps(rating_vec, enc_w, enc_b, dec_w, dec_b, before_score, piw,
                 batch_idx, inter_idx, lengths):
    """Host-side sharding / layout prep. Index arithmetic + casts only."""
    f32 = np.float32
    rating_vec = np.asarray(rating_vec, f32)
    enc_w = np.asarray(enc_w, f32)
    enc_b = np.asarray(enc_b, f32)
    dec_w = np.asarray(dec_w, f32)
    dec_b = np.asarray(dec_b, f32)
    before_score = np.asarray(before_score, f32)
    piw = np.asarray(piw, f32)
    batch_idx = np.asarray(batch_idx)
    inter_idx = np.asarray(inter_idx)
    lengths = np.asarray(lengths)

    # shared (replicated) tensors
    xmat = np.zeros((KT * 128, 402), f32)
    xmat[:I, 0:D] = enc_w.T
    xmat[:I, D:2 * D] = dec_w
    xmat[:I, 2 * D] = dec_b
    xmat[:I, 2 * D + 1] = 1.0
    xmat[I, 0:D] = enc_b
    xmat_bf = np.ascontiguousarray(
        xmat.astype(_BF16).reshape(NG, KG, 128, 402).transpose(0, 2, 1, 3))

    dwt = dec_w.T  # [200, 20000]
    dwta = np.ascontiguousarray(dwt[:128]).astype(_BF16)
    dwtb = np.concatenate([dwt[128:D], dec_b[None, :]], axis=0).astype(_BF16)

    ident = np.eye(128, dtype=_BF16)

    in_maps = []
    for c in range(NCORES):
        r0 = BSH * c
        ratingT = np.zeros((KT * 128, BSH), f32)
        ratingT[:I] = rating_vec[r0:r0 + BSH].T
        ratingT[I] = 1.0

        users = np.nonzero((batch_idx >= r0) & (batch_idx < r0 + BSH))[0]
        nu = len(users)
        assert nu <= UCAP, f"core {c}: {nu} users > capacity {UCAP}"

        bef = np.empty((UCAP, I), _BF16)
        bef[:nu] = before_score[users]
        bef[nu:] = before_score[0]

        gidx, vm, par, ipar = _pack_gather(users, inter_idx, lengths)

        onehot_arr = np.zeros((128, UCAP), f32)
        onehot_arr[batch_idx[users] - r0, np.arange(nu)] = 1.0

        piw_arr = np.zeros((UCAP, 1), f32)
        piw_arr[:nu, 0] = piw[users]
        len_arr = np.ones((UCAP, 1), f32)
        len_arr[:nu, 0] = lengths[users].astype(f32)

        blob = np.zeros((128, BLOB_BYTES), np.uint8)

        def put(name, arr):
            off, sz = BLOB_LAYOUT[name]
            bview = np.ascontiguousarray(arr).view(np.uint8).reshape(128, sz)
            blob[:, off:off + sz] = bview

        put("vm0", vm[0]); put("vm1", vm[1])
        put("par0", par[0]); put("par1", par[1])
        put("ipar0", ipar[0]); put("ipar1", ipar[1])
        put("gidx0", gidx[0]); put("gidx1", gidx[1])
        put("onehot", onehot_arr)
        put("ident", ident)
        put("piw", piw_arr)
        put("len", len_arr)

        in_maps.append(dict(
            ratingT=np.ascontiguousarray(
                ratingT.astype(_BF16).reshape(NG, KG, 128, BSH)
                .transpose(0, 2, 1, 3)),
            xmat=xmat_bf,
            dwta=dwta,
            dwtb=dwtb,
            bef_0=np.ascontiguousarray(bef[:, :IHALF]),
            bef_1=np.ascontiguousarray(bef[:, IHALF:]),
            blob=blob,
        ))
    return in_maps


def combine(outs):
    base = f32sum(o[0, 0] for o in outs)
    kl = f32sum(o[0, 1] for o in outs)
    return np.float32(base), np.float32(kl)


def f32sum(it):
    acc = np.float32(0.0)
    for v in it:
        acc = np.float32(acc + np.float32(v))
    return acc


def kernel(**inputs):
    nc = get_program()
    in_maps = make_in_maps(**inputs)
    from concourse.bass_utils import run_bass_kernel_spmd
    res = run_bass_kernel_spmd(nc, in_maps, list(range(NCORES)))
    outs = [res.results[c]["out"] for c in range(NCORES)]
    return combine(outs)


# revision 39
# speedup vs baseline: 1.1812x; 1.0428x over previous
"""Trainium2 Bass kernel for nn_CL_VAE (Multi-VAE loss + contrastive-learning KL).

Strategy (8 NeuronCores, data-parallel over batch rows + common users):
  - core c owns batch rows [128c, 128c+128) and the common users whose
    batch_idx falls in that range (padded to 128 user slots, user = SBUF
    partition).
  - Host prep is layout-only: shard/transpose/concat/cast inputs, build
    index tables + 0/1 masks from the integer index inputs.
  - Device per core:
      phase 0: gather before_score at (user, inter_idx) positions with
               gpsimd ap_gather (ragged-packed word indices + parity
               select, split by item halves), w = b_sel at valid slots,
               S0 = sum w, S1 = sum w*log(b_sel).
      phase 1: one fused matmul over K = I(+1 bias row):
               [h_pre | rating@dec_w | rating@dec_b | sum(rating)] =
               rating_aug^T.T @ [enc_w^T | dec_w | dec_b | 1].
      phase 2: decoder matmuls logits = [h|1] @ [dec_w^T; dec_b] for both
               batch rows and user rows from resident weights, high item
               half first (so its user-logit gather overlaps the low
               half); exp+accumulate for softmax denominators.
      phase 3: lse = log(sum exp), gather user logits, S2 = sum(w*glog),
               combine into the two loss partials, reduce over partitions
               with a ones-matmul.
  - Host combines the 8 per-core [1,2] partials by summation only.

log-softmax identity used (exact in real arithmetic):
  sum_i log_probs*rating = sum_i logits*rating - lse*sum_i rating
  log(p_sel) = logits_sel - lse      (since p_sel = exp(log_probs))
The +EPS terms of the reference cancel in (log_b - log_p) and are below
fp32 resolution elsewhere.

Gathers run on bf16 data but ap_gather moves 4-byte words, so indices
are word indices (il>>1) and a host-provided parity mask pair selects
the bf16 half: val = ipar*even + par*odd.
"""

import numpy as np
import ml_dtypes

# ---- hardcoded problem shapes ----
B, I, D, U, L = 1024, 20000, 200, 512, 200
NCORES = 8
BSH = B // NCORES      # 128 batch rows per core
UCAP = 128             # padded user slots per core
KTOT = I + 1           # contraction length (+1 bias/ones row)
KG = 8                 # k-tiles per DMA batch in phase 1
KT = 160               # k-tiles (zero-padded up from 157)
NG = KT // KG          # 20 phase-1 DMA groups
CH = 500               # decoder free-dim chunk
NCH = I // CH          # 40
IHALF = I // 2         # item split for gather overlap (10000)
NCHH = NCH // 2        # 20 chunks per half
NIDX = 1440            # ragged-packed gather indices per 16-part group/half

WSCALE = 32.0          # power-of-two weight prescale for fp8 encoding

# packed small-constant blob layout: per-partition byte (offset, size)
_B2 = NIDX * 2
BLOB_LAYOUT = {
    "vm0": (0, _B2), "vm1": (_B2, _B2),
    "par0": (2 * _B2, _B2), "par1": (3 * _B2, _B2),
    "ipar0": (4 * _B2, _B2), "ipar1": (5 * _B2, _B2),
    "gidx0": (6 * _B2, NIDX // 8), "gidx1": (6 * _B2 + NIDX // 8, NIDX // 8),
    "onehot": (6 * _B2 + NIDX // 4, 512),
    "ident": (6 * _B2 + NIDX // 4 + 512, 256),
    "piw": (6 * _B2 + NIDX // 4 + 768, 4),
    "len": (6 * _B2 + NIDX // 4 + 772, 4),
}
BLOB_BYTES = 6 * _B2 + NIDX // 4 + 776

_BF16 = ml_dtypes.bfloat16
_F8 = ml_dtypes.float8_e4m3

_prog_cache = {}


def _build_program():
    import concourse.bacc as bacc
    import concourse.mybir as mybir
    import concourse.tile as tile
    from contextlib import ExitStack

    f32 = mybir.dt.float32
    bf16 = mybir.dt.bfloat16
    i16 = mybir.dt.int16
    Act = mybir.ActivationFunctionType
    Alu = mybir.AluOpType

    nc = bacc.Bacc("TRN2", target_bir_lowering=False, debug=False,
                   num_devices=NCORES)

    f8 = mybir.dt.float8e4
    u8_ = mybir.dt.uint8
    ratingT = nc.dram_tensor("ratingT", [128, KT * BSH], u8_,
                             kind="ExternalInput")
    xmat = nc.dram_tensor("xmat", [NG, 128, KG, 402], u8_,
                          kind="ExternalInput")
    dwta_d = nc.dram_tensor("dwta", [128, I], u8_, kind="ExternalInput")
    dwtb_d = nc.dram_tensor("dwtb", [73, I], u8_, kind="ExternalInput")
    bef_d = [nc.dram_tensor(f"bef_{h}", [UCAP, IHALF], bf16,
                            kind="ExternalInput") for h in range(2)]
    u8 = mybir.dt.uint8
    blob_d = nc.dram_tensor("blob", [128, BLOB_BYTES], u8,
                            kind="ExternalInput")
    out_d = nc.dram_tensor("out", [1, 2], f32, kind="ExternalOutput")

    with ExitStack() as ctx:
        tc = ctx.enter_context(tile.TileContext(nc))
        pool = ctx.enter_context(tc.tile_pool(name="main", bufs=1))
        big = ctx.enter_context(tc.tile_pool(name="big", bufs=1))
        gpool = ctx.enter_context(tc.tile_pool(name="gat", bufs=1))
        stream = ctx.enter_context(tc.tile_pool(name="stream", bufs=3))
        psum = ctx.enter_context(tc.tile_pool(name="ps", bufs=1, space="PSUM"))
        psmm = ctx.enter_context(tc.tile_pool(name="psmm", bufs=2,
                                              space="PSUM"))

        def parity_select(gw, par_ap, ipar_ap, out_dtype, name):
            """val = ipar*even(gw) + par*odd(gw); returns [128, NIDX] tile."""
            wb = gw[:].bitcast(bf16).rearrange("p (j t) -> p j t", t=2)
            t_e = gpool.tile([128, NIDX], out_dtype, tag="val", bufs=2,
                             name=f"{name}_e")
            nc.vector.tensor_mul(t_e[:], ipar_ap, wb[:, :, 0:1])
            t_o = gpool.tile([128, NIDX], out_dtype, tag="val", bufs=2,
                             name=f"{name}_o")
            nc.vector.tensor_mul(t_o[:], par_ap, wb[:, :, 1:2])
            val = gpool.tile([128, NIDX], out_dtype, tag="val2", bufs=2,
                             name=f"{name}_v")
            nc.vector.tensor_add(val[:], t_e[:], t_o[:])
            return val

        # ------- phase 1 main matmul stream (DMA priority: first) -------
        rt_res = pool.tile([128, KT * BSH], mybir.dt.uint8)
        nc.scalar.dma_start(out=rt_res[:, :KT * BSH // 2],
                            in_=ratingT[:, :KT * BSH // 2])
        nc.scalar.dma_start(out=rt_res[:, KT * BSH // 2:],
                            in_=ratingT[:, KT * BSH // 2:])
        rt8 = rt_res[:].bitcast(f8)
        ps1 = psmm.tile([128, 402], f32, tag="mm")
        for g in range(NG):
            x_g = stream.tile([128, KG, 402], mybir.dt.uint8, tag="xk",
                              bufs=6)
            nc.sync.dma_start(out=x_g[:], in_=xmat[g])
            x8 = x_g[:].bitcast(f8)
            for kk in range(KG):
                k = KG * g + kk
                nc.tensor.matmul(ps1[:], rt8[:, BSH * k:BSH * (k + 1)],
                                 x8[:, kk, :],
                                 start=(k == 0), stop=(k == KT - 1))

        # ------- scalar-queue traffic: dwt, bef; sync adds blob at end ----
        dwta_t = pool.tile([128, I], mybir.dt.uint8, name="dwta")
        dwtb_t = pool.tile([73, I], mybir.dt.uint8, name="dwtb")
        dwta = dwta_t[:].bitcast(f8)
        dwtb = dwtb_t[:].bitcast(f8)
        nc.scalar.dma_start(out=dwta_t[:, IHALF:I], in_=dwta_d[:, IHALF:I])
        nc.scalar.dma_start(out=dwtb_t[:, IHALF:I], in_=dwtb_d[:, IHALF:I])
        nc.scalar.dma_start(out=dwta_t[:, 0:IHALF], in_=dwta_d[:, 0:IHALF])
        nc.scalar.dma_start(out=dwtb_t[:, 0:IHALF], in_=dwtb_d[:, 0:IHALF])
        bef = [None, None]
        for h in (1, 0):
            bt = big.tile([UCAP, IHALF], bf16, tag=f"b{h}", name=f"bef{h}")
            nc.scalar.dma_start(out=bt[:], in_=bef_d[h][:])
            bef[h] = bt
        blob_sb = pool.tile([128, BLOB_BYTES], mybir.dt.uint8)
        nc.sync.dma_start(out=blob_sb[:], in_=blob_d[:])

        def _bview(name, dtype):
            off, sz = BLOB_LAYOUT[name]
            return blob_sb[:, off:off + sz].bitcast(dtype)

        vm = [_bview("vm0", bf16), _bview("vm1", bf16)]
        par = [_bview("par0", bf16), _bview("par1", bf16)]
        ipar = [_bview("ipar0", bf16), _bview("ipar1", bf16)]
        gidx = [_bview("gidx0", i16), _bview("gidx1", i16)]
        onehot_sb = _bview("onehot", f32)
        ident_sb = _bview("ident", bf16)
        piw_sb = _bview("piw", f32)
        len_sb = _bview("len", f32)

        # ------- phase 0: before_score gathers (idle gpsimd, early) -------
        s0 = pool.tile([128, 1], f32)
        s1 = pool.tile([128, 1], f32)
        w_t = [None, None]
        for h in (1, 0):
            gw = gpool.tile([128, NIDX], f32, tag="tmp", bufs=2,
                            name=f"bw{h}")
            nc.gpsimd.ap_gather(gw[:], bef[h][:].bitcast(f32), gidx[h],
                                channels=128, num_elems=IHALF // 2, d=1,
                                num_idxs=NIDX)
            val = parity_select(gw, par[h], ipar[h], bf16, f"bv{h}")
            logb = gpool.tile([128, NIDX], f32, tag="tmp", bufs=2,
                              name=f"logb{h}")
            nc.scalar.activation(logb[:], val[:], Act.Ln)
            wt = pool.tile([128, NIDX], f32, name=f"w{h}")
            nc.vector.tensor_mul(wt[:], vm[h], val[:])
            w_t[h] = wt
            s0h = pool.tile([128, 1], f32, tag="s0h", bufs=2)
            nc.vector.tensor_reduce(s0h[:], wt[:], axis=mybir.AxisListType.X,
                                    op=Alu.add)
            scr_a = gpool.tile([128, NIDX], f32, tag="tmp", bufs=2,
                               name=f"wl{h}")
            nc.vector.tensor_mul(scr_a[:], wt[:], logb[:])
            s1h = pool.tile([128, 1], f32, tag="s1h", bufs=2)
            nc.vector.tensor_reduce(s1h[:], scr_a[:],
                                    axis=mybir.AxisListType.X, op=Alu.add)
            if h == 1:
                nc.vector.tensor_copy(s0[:], s0h[:])
                nc.vector.tensor_copy(s1[:], s1h[:])
            else:
                nc.vector.tensor_add(s0[:], s0[:], s0h[:])
                nc.vector.tensor_add(s1[:], s1[:], s1h[:])

        # ------- phase 1 epilogue -------
        h_f = pool.tile([128, D], f32)
        nc.scalar.activation(h_f[:], ps1[:, 0:D], Act.Tanh, scale=1.0 / WSCALE)
        h_bf = pool.tile([128, D + 1], bf16)
        nc.vector.tensor_copy(h_bf[:, 0:D], h_f[:])
        nc.vector.memset(h_bf[:, D:D + 1], 1.0)
        scr200 = pool.tile([128, D], f32)
        nc.vector.tensor_mul(scr200[:], h_f[:], ps1[:, D:2 * D])
        dot_p = pool.tile([128, 1], f32)
        nc.vector.tensor_reduce(dot_p[:], scr200[:], axis=mybir.AxisListType.X,
                                op=Alu.add)
        dot_sc = pool.tile([128, 1], f32)
        nc.vector.tensor_add(dot_sc[:], dot_p[:], ps1[:, 2 * D:2 * D + 1])
        dot_row = pool.tile([128, 1], f32)
        nc.vector.tensor_scalar_mul(dot_row[:], dot_sc[:], 1.0 / WSCALE)
        rsum = pool.tile([128, 1], f32)
        nc.vector.tensor_copy(rsum[:], ps1[:, 2 * D + 1:2 * D + 2])

        ps_hu = psum.tile([128, D], f32, tag="sm1")
        nc.tensor.matmul(ps_hu[:], onehot_sb, h_f[:], start=True, stop=True)
        hu_bf = pool.tile([128, D + 1], bf16)
        nc.vector.tensor_copy(hu_bf[:, 0:D], ps_hu[:])
        nc.vector.memset(hu_bf[:, D:D + 1], 1.0)

        # transposes: h^T and h_u^T, each augmented with a trailing ones row
        hta = pool.tile([128, BSH], f8)
        htb = pool.tile([73, BSH], f8)
        huta = pool.tile([128, UCAP], f8)
        hutb = pool.tile([73, UCAP], f8)
        for src, dsta, dstb in ((h_bf, hta, htb), (hu_bf, huta, hutb)):
            ps_t1 = psum.tile([128, 128], bf16, tag="sm2", bufs=2)
            nc.tensor.transpose(ps_t1[:], src[:, 0:128], ident_sb)
            nc.vector.tensor_copy(dsta[:], ps_t1[:])
            ps_t2 = psum.tile([73, 128], bf16, tag="sm2", bufs=2)
            nc.tensor.transpose(ps_t2[:], src[:, 128:D + 1], ident_sb)
            nc.vector.tensor_copy(dstb[:], ps_t2[:])

        # ------- phase 2: decoder stream, high item half first -------
        ulog = [None, None]
        ulog[1] = big.tile([UCAP, IHALF], bf16, tag="b1", name="ulog1")
        ulog[0] = big.tile([UCAP, IHALF], bf16, tag="b0", name="ulog0")
        ssum = pool.tile([128, NCH], f32)
        est = stream.tile([128, CH], f32, tag="est", bufs=2)
        for ci in range(NCH):
            c = ci + NCHH if ci < NCHH else ci - NCHH  # hi half first
            das = dwta[:, CH * c:CH * (c + 1)]
            dbs = dwtb[:, CH * c:CH * (c + 1)]
            pm = psmm.tile([128, CH], f32, tag="mm")
            nc.tensor.matmul(pm[:], hta[:], das, start=True, stop=False)
            nc.tensor.matmul(pm[:], htb[:], dbs, start=False, stop=True)
            pu = psmm.tile([128, CH], f32, tag="pu")
            nc.tensor.matmul(pu[:], huta[:], das, start=True, stop=False)
            nc.tensor.matmul(pu[:], hutb[:], dbs, start=False, stop=True)
            nc.scalar.activation(est[:], pm[:], Act.Exp,
                                 accum_out=ssum[:, c:c + 1],
                                 scale=1.0 / WSCALE)
            half = 1 if c >= NCHH else 0
            c0 = CH * (c - NCHH) if c >= NCHH else CH * c
            nc.vector.tensor_scalar_mul(ulog[half][:, c0:c0 + CH], pu[:],
                                        1.0 / WSCALE)

        # ------- phase 3: lse, user-logit gathers, combine -------
        s2 = pool.tile([128, 1], f32)
        for h in (1, 0):
            gw = gpool.tile([128, NIDX], f32, tag="tmp", bufs=2,
                            name=f"uw{h}")
            nc.gpsimd.ap_gather(gw[:], ulog[h][:].bitcast(f32), gidx[h],
                                channels=128, num_elems=IHALF // 2, d=1,
                                num_idxs=NIDX)
            valg = parity_select(gw, par[h], ipar[h], f32, f"uv{h}")
            scr = gpool.tile([128, NIDX], f32, tag="tmp", bufs=2,
                             name=f"ws{h}")
            nc.vector.tensor_mul(scr[:], w_t[h][:], valg[:])
            s2h = pool.tile([128, 1], f32, tag="s2h", bufs=2)
            nc.vector.tensor_reduce(s2h[:], scr[:], axis=mybir.AxisListType.X,
                                    op=Alu.add)
            if h == 1:
                nc.vector.tensor_copy(s2[:], s2h[:])
            else:
                nc.vector.tensor_add(s2[:], s2[:], s2h[:])

        s_tot = pool.tile([128, 1], f32)
        nc.vector.tensor_reduce(s_tot[:], ssum[:], axis=mybir.AxisListType.X,
                                op=Alu.add)
        lse = pool.tile([128, 1], f32)
        nc.scalar.activation(lse[:], s_tot[:], Act.Ln)

        ps_ls = psum.tile([128, 1], f32, tag="sm1")
        nc.tensor.matmul(ps_ls[:], onehot_sb, lse[:], start=True, stop=True)

        invlen = pool.tile([128, 1], f32)
        nc.vector.reciprocal(invlen[:], len_sb)

        # kl_slot = (s1 - s2 + lse_u*s0) * piw * invlen / U
        t0 = pool.tile([128, 1], f32)
        nc.vector.tensor_sub(t0[:], s1[:], s2[:])
        t1 = pool.tile([128, 1], f32)
        nc.vector.tensor_mul(t1[:], ps_ls[:], s0[:])
        t2 = pool.tile([128, 1], f32)
        nc.vector.tensor_add(t2[:], t0[:], t1[:])
        t3 = pool.tile([128, 1], f32)
        nc.vector.tensor_mul(t3[:], t2[:], piw_sb)
        t4 = pool.tile([128, 1], f32)
        nc.vector.tensor_mul(t4[:], t3[:], invlen[:])
        pair = pool.tile([128, 2], f32)
        nc.vector.tensor_scalar_mul(pair[:, 1:2], t4[:], 1.0 / U)

        # base_row = -(dot_row - lse*rsum)/B
        t5 = pool.tile([128, 1], f32)
        nc.vector.tensor_mul(t5[:], lse[:], rsum[:])
        t6 = pool.tile([128, 1], f32)
        nc.vector.tensor_sub(t6[:], dot_row[:], t5[:])
        nc.vector.tensor_scalar_mul(pair[:, 0:1], t6[:], -1.0 / B)

        ones1 = pool.tile([128, 1], f32)
        nc.vector.memset(ones1[:], 1.0)
        ps_fin = psum.tile([1, 2], f32, tag="sm2", bufs=2)
        nc.tensor.matmul(ps_fin[:], ones1[:], pair[:], start=True, stop=True)
        out_sb = pool.tile([1, 2], f32)
        nc.vector.tensor_copy(out_sb[:], ps_fin[:])
        nc.sync.dma_start(out=out_d[:], in_=out_sb[:])

    nc.compile()
    return nc


def get_program():
    if "nc" not in _prog_cache:
        _prog_cache["nc"] = _build_program()
    return _prog_cache["nc"]


def _pack_gather(users, inter_idx, lengths):
    """Ragged-pack per-group gather word indices, split at IHALF.

    Returns per half: wrapped int16 word-index array [128, NIDX//16],
    valid mask vm, parity mask par, inverse-parity mask ipar (all
    [128, NIDX]; ipar=1 at padding so the selected value stays > 0).
    """
    gidx = [np.zeros((128, NIDX // 16), np.int16) for _ in range(2)]
    vm = [np.zeros((128, NIDX), _BF16) for _ in range(2)]
    par = [np.zeros((128, NIDX), _BF16) for _ in range(2)]
    ipar = [np.ones((128, NIDX), _BF16) for _ in range(2)]
    jj = np.arange(NIDX)
    for g in range(8):
        us = users[16 * g:16 * (g + 1)]
        lists = [np.zeros(NIDX, np.int64) for _ in range(2)]
        pos = [0, 0]
        for s_loc, u in enumerate(us):
            il = inter_idx[u][:lengths[u]].astype(np.int64)
            for half, sel in enumerate((il < IHALF, il >= IHALF)):
                idx_h = il[sel] - half * IHALF
                n = len(idx_h)
                p0 = pos[half]
                assert p0 + n <= NIDX, "gather capacity exceeded"
                lists[half][p0:p0 + n] = idx_h >> 1
                rows = 16 * g + s_loc
                vm[half][rows, p0:p0 + n] = 1.0
                odd = (idx_h & 1).astype(_BF16)
                par[half][rows, p0:p0 + n] = odd
                # ipar defaults to 1 (padding-safe); overwrite real slots
                ipar[half][rows, p0:p0 + n] = 1.0 - odd.astype(np.float32)
                pos[half] += n
        for half in range(2):
            # positions are packed per group: every partition of the group
            # shares the same index list, wrapped across 16 partitions
            gidx[half][16 * g + (jj % 16), jj // 16] = lists[half][jj]
    # ipar rows for positions claimed by OTHER partitions in the group must
    # stay consistent with par: val is only consumed where vm=1, but keep
    # par+ipar <= 1 to avoid overflow concerns; nothing else needed.
    return gidx, vm, par, ipar


def make_in_maps(rating_vec, enc_w, enc_b, dec_w, dec_b, before_score, piw,
                 batch_idx, inter_idx, lengths):
    """Host-side sharding / layout prep. Index arithmetic + casts only."""
    f32 = np.float32
    rating_vec = np.asarray(rating_vec, f32)
    enc_w = np.asarray(enc_w, f32)
    enc_b = np.asarray(enc_b, f32)
    dec_w = np.asarray(dec_w, f32)
    dec_b = np.asarray(dec_b, f32)
    before_score = np.asarray(before_score, f32)
    piw = np.asarray(piw, f32)
    batch_idx = np.asarray(batch_idx)
    inter_idx = np.asarray(inter_idx)
    lengths = np.asarray(lengths)

    # shared (replicated) tensors; weights prescaled by WSCALE (power of
    # two, exactly undone on device) so fp8 encoding stays in normal range
    xmat = np.zeros((KT * 128, 402), f32)
    xmat[:I, 0:D] = enc_w.T
    xmat[:I, D:2 * D] = dec_w
    xmat[:I, 2 * D] = dec_b
    xmat[I, 0:D] = enc_b
    xmat[:, 0:2 * D + 1] *= WSCALE
    xmat[:I, 2 * D + 1] = 1.0
    xmat_bf = np.ascontiguousarray(
        xmat.astype(_F8).reshape(NG, KG, 128, 402)
        .transpose(0, 2, 1, 3)).view(np.uint8)

    dwt = dec_w.T * WSCALE  # [200, 20000]
    dwta = np.ascontiguousarray(dwt[:128]).astype(_F8).view(np.uint8)
    dwtb = np.concatenate([dwt[128:D], dec_b[None, :] * WSCALE],
                          axis=0).astype(_F8).view(np.uint8)

    ident = np.eye(128, dtype=_BF16)

    in_maps = []
    for c in range(NCORES):
        r0 = BSH * c
        ratingT = np.zeros((KT * 128, BSH), f32)
        ratingT[:I] = rating_vec[r0:r0 + BSH].T
        ratingT[I] = 1.0

        users = np.nonzero((batch_idx >= r0) & (batch_idx < r0 + BSH))[0]
        nu = len(users)
        assert nu <= UCAP, f"core {c}: {nu} users > capacity {UCAP}"

        bef = np.empty((UCAP, I), _BF16)
        bef[:nu] = before_score[users]
        bef[nu:] = before_score[0]

        gidx, vm, par, ipar = _pack_gather(users, inter_idx, lengths)

        onehot_arr = np.zeros((128, UCAP), f32)
        onehot_arr[batch_idx[users] - r0, np.arange(nu)] = 1.0

        piw_arr = np.zeros((UCAP, 1), f32)
        piw_arr[:nu, 0] = piw[users]
        len_arr = np.ones((UCAP, 1), f32)
        len_arr[:nu, 0] = lengths[users].astype(f32)

        blob = np.zeros((128, BLOB_BYTES), np.uint8)

        def put(name, arr):
            off, sz = BLOB_LAYOUT[name]
            bview = np.ascontiguousarray(arr).view(np.uint8).reshape(128, sz)
            blob[:, off:off + sz] = bview

        put("vm0", vm[0]); put("vm1", vm[1])
        put("par0", par[0]); put("par1", par[1])
        put("ipar0", ipar[0]); put("ipar1", ipar[1])
        put("gidx0", gidx[0]); put("gidx1", gidx[1])
        put("onehot", onehot_arr)
        put("ident", ident)
        put("piw", piw_arr)
        put("len", len_arr)

        in_maps.append(dict(
            ratingT=np.ascontiguousarray(
                ratingT.astype(_F8).reshape(KT, 128, BSH)
                .transpose(1, 0, 2)).reshape(128, KT * BSH).view(np.uint8),
            xmat=xmat_bf,
            dwta=dwta,
            dwtb=dwtb,
            bef_0=np.ascontiguousarray(bef[:, :IHALF]),
            bef_1=np.ascontiguousarray(bef[:, IHALF:]),
            blob=blob,
        ))
    return in_maps


def combine(outs):
    base = f32sum(o[0, 0] for o in outs)
    kl = f32sum(o[0, 1] for o in outs)
    return np.float32(base), np.float32(kl)


def f32sum(it):
    acc = np.float32(0.0)
    for v in it:
        acc = np.float32(acc + np.float32(v))
    return acc


def kernel(**inputs):
    nc = get_program()
    in_maps = make_in_maps(**inputs)
    from concourse.bass_utils import run_bass_kernel_spmd
    res = run_bass_kernel_spmd(nc, in_maps, list(range(NCORES)))
    outs = [res.results[c]["out"] for c in range(NCORES)]
    return combine(outs)


# revision 41
# speedup vs baseline: 2.1840x; 1.8490x over previous
"""Trainium2 Bass kernel for nn_CL_VAE (Multi-VAE loss + contrastive-learning KL).

Strategy (8 NeuronCores, data-parallel over batch rows + common users):
  - core c owns batch rows [128c, 128c+128) and the common users whose
    batch_idx falls in that range (padded to 128 user slots, user = SBUF
    partition).
  - Host prep is layout-only: shard/transpose/concat/cast inputs, build
    index tables + 0/1 masks from the integer index inputs.
  - Device per core:
      phase 0: gather before_score at (user, inter_idx) positions with
               gpsimd ap_gather (ragged-packed word indices + parity
               select, split by item halves), w = b_sel at valid slots,
               S0 = sum w, S1 = sum w*log(b_sel).
      phase 1: one fused matmul over K = I(+1 bias row):
               [h_pre | rating@dec_w | rating@dec_b | sum(rating)] =
               rating_aug^T.T @ [enc_w^T | dec_w | dec_b | 1].
      phase 2: decoder matmuls logits = [h|1] @ [dec_w^T; dec_b] for both
               batch rows and user rows from resident weights, high item
               half first (so its user-logit gather overlaps the low
               half); exp+accumulate for softmax denominators.
      phase 3: lse = log(sum exp), gather user logits, S2 = sum(w*glog),
               combine into the two loss partials, reduce over partitions
               with a ones-matmul.
  - Host combines the 8 per-core [1,2] partials by summation only.

log-softmax identity used (exact in real arithmetic):
  sum_i log_probs*rating = sum_i logits*rating - lse*sum_i rating
  log(p_sel) = logits_sel - lse      (since p_sel = exp(log_probs))
The +EPS terms of the reference cancel in (log_b - log_p) and are below
fp32 resolution elsewhere.

Gathers run on bf16 data but ap_gather moves 4-byte words, so indices
are word indices (il>>1) and a host-provided parity mask pair selects
the bf16 half: val = ipar*even + par*odd.
"""

import numpy as np
import ml_dtypes

# ---- hardcoded problem shapes ----
B, I, D, U, L = 1024, 20000, 200, 512, 200
NCORES = 8
BSH = B // NCORES      # 128 batch rows per core
UCAP = 128             # padded user slots per core
KTOT = I + 1           # contraction length (+1 bias/ones row)
KG = 8                 # k-tiles per DMA batch in phase 1
KT = 160               # k-tiles (zero-padded up from 157)
NG = KT // KG          # 20 phase-1 DMA groups
CH = 500               # decoder free-dim chunk
NCH = I // CH          # 40
IHALF = I // 2         # item split for gather overlap (10000)
NCHH = NCH // 2        # 20 chunks per half
NIDX = 768             # ragged-packed gather indices per 16-part group/half

WSCALE = 32.0          # power-of-two weight prescale for fp8 encoding

# packed small-constant blob layout: per-partition byte (offset, size)
_B2 = NIDX * 2
BLOB_LAYOUT = {
    "vm0": (0, _B2), "vm1": (_B2, _B2),
    "par0": (2 * _B2, _B2), "par1": (3 * _B2, _B2),
    "ipar0": (4 * _B2, _B2), "ipar1": (5 * _B2, _B2),
    "gidx0": (6 * _B2, NIDX // 8), "gidx1": (6 * _B2 + NIDX // 8, NIDX // 8),
    "onehot": (6 * _B2 + NIDX // 4, 512),
    "ident": (6 * _B2 + NIDX // 4 + 512, 256),
    "piw": (6 * _B2 + NIDX // 4 + 768, 4),
    "len": (6 * _B2 + NIDX // 4 + 772, 4),
}
BLOB_BYTES = 6 * _B2 + NIDX // 4 + 776

_BF16 = ml_dtypes.bfloat16
_F8 = ml_dtypes.float8_e4m3

_prog_cache = {}


def _build_program():
    import concourse.bacc as bacc
    import concourse.mybir as mybir
    import concourse.tile as tile
    from contextlib import ExitStack

    f32 = mybir.dt.float32
    bf16 = mybir.dt.bfloat16
    i16 = mybir.dt.int16
    Act = mybir.ActivationFunctionType
    Alu = mybir.AluOpType

    nc = bacc.Bacc("TRN2", target_bir_lowering=False, debug=False,
                   num_devices=NCORES)

    f8 = mybir.dt.float8e4
    u8_ = mybir.dt.uint8
    ratingT = nc.dram_tensor("ratingT", [128, KT * BSH], u8_,
                             kind="ExternalInput")
    xmat = nc.dram_tensor("xmat", [NG, 128, KG, 402], u8_,
                          kind="ExternalInput")
    dwta_d = nc.dram_tensor("dwta", [128, I], u8_, kind="ExternalInput")
    dwtb_d = nc.dram_tensor("dwtb", [73, I], u8_, kind="ExternalInput")
    bef_d = [nc.dram_tensor(f"bef_{h}", [UCAP, IHALF], bf16,
                            kind="ExternalInput") for h in range(2)]
    u8 = mybir.dt.uint8
    blob_d = nc.dram_tensor("blob", [128, BLOB_BYTES], u8,
                            kind="ExternalInput")
    out_d = nc.dram_tensor("out", [1, 2], f32, kind="ExternalOutput")

    with ExitStack() as ctx:
        tc = ctx.enter_context(tile.TileContext(nc))
        pool = ctx.enter_context(tc.tile_pool(name="main", bufs=1))
        big = ctx.enter_context(tc.tile_pool(name="big", bufs=1))
        gpool = ctx.enter_context(tc.tile_pool(name="gat", bufs=1))
        stream = ctx.enter_context(tc.tile_pool(name="stream", bufs=3))
        psum = ctx.enter_context(tc.tile_pool(name="ps", bufs=1, space="PSUM"))
        psmm = ctx.enter_context(tc.tile_pool(name="psmm", bufs=2,
                                              space="PSUM"))

        def parity_select(gw, par_ap, ipar_ap, out_dtype, name):
            """val = ipar*even(gw) + par*odd(gw); returns [128, NIDX] tile."""
            wb = gw[:].bitcast(bf16).rearrange("p (j t) -> p j t", t=2)
            t_e = gpool.tile([128, NIDX], out_dtype, tag="val", bufs=2,
                             name=f"{name}_e")
            nc.vector.tensor_mul(t_e[:], ipar_ap, wb[:, :, 0:1])
            t_o = gpool.tile([128, NIDX], out_dtype, tag="val", bufs=2,
                             name=f"{name}_o")
            nc.vector.tensor_mul(t_o[:], par_ap, wb[:, :, 1:2])
            val = gpool.tile([128, NIDX], out_dtype, tag="val2", bufs=2,
                             name=f"{name}_v")
            nc.vector.tensor_add(val[:], t_e[:], t_o[:])
            return val

        # ------- phase 1 main matmul stream (DMA priority: first) -------
        rt_res = pool.tile([128, KT * BSH], mybir.dt.uint8)
        nc.scalar.dma_start(out=rt_res[:, :KT * BSH // 2],
                            in_=ratingT[:, :KT * BSH // 2])
        nc.scalar.dma_start(out=rt_res[:, KT * BSH // 2:],
                            in_=ratingT[:, KT * BSH // 2:])
        rt8 = rt_res[:].bitcast(f8)
        ps1 = psmm.tile([128, 402], f32, tag="mm")
        for g in range(NG):
            x_g = stream.tile([128, KG, 402], mybir.dt.uint8, tag="xk",
                              bufs=6)
            nc.sync.dma_start(out=x_g[:], in_=xmat[g])
            x8 = x_g[:].bitcast(f8)
            for kk in range(KG):
                k = KG * g + kk
                nc.tensor.matmul(ps1[:], rt8[:, BSH * k:BSH * (k + 1)],
                                 x8[:, kk, :],
                                 start=(k == 0), stop=(k == KT - 1))

        # ------- scalar-queue traffic: blob+bef early, then dwt -------
        blob_sb = pool.tile([128, BLOB_BYTES], mybir.dt.uint8)
        nc.scalar.dma_start(out=blob_sb[:], in_=blob_d[:])
        bef = [None, None]
        for h in (1, 0):
            bt = big.tile([UCAP, IHALF], bf16, tag=f"b{h}", name=f"bef{h}")
            nc.scalar.dma_start(out=bt[:], in_=bef_d[h][:])
            bef[h] = bt
        dwta_t = pool.tile([128, I], mybir.dt.uint8, name="dwta")
        dwtb_t = pool.tile([73, I], mybir.dt.uint8, name="dwtb")
        dwta = dwta_t[:].bitcast(f8)
        dwtb = dwtb_t[:].bitcast(f8)
        nc.scalar.dma_start(out=dwta_t[:, IHALF:I], in_=dwta_d[:, IHALF:I])
        nc.scalar.dma_start(out=dwtb_t[:, IHALF:I], in_=dwtb_d[:, IHALF:I])
        nc.scalar.dma_start(out=dwta_t[:, 0:IHALF], in_=dwta_d[:, 0:IHALF])
        nc.scalar.dma_start(out=dwtb_t[:, 0:IHALF], in_=dwtb_d[:, 0:IHALF])

        def _bview(name, dtype):
            off, sz = BLOB_LAYOUT[name]
            return blob_sb[:, off:off + sz].bitcast(dtype)

        vm = [_bview("vm0", bf16), _bview("vm1", bf16)]
        par = [_bview("par0", bf16), _bview("par1", bf16)]
        ipar = [_bview("ipar0", bf16), _bview("ipar1", bf16)]
        gidx = [_bview("gidx0", i16), _bview("gidx1", i16)]
        onehot_sb = _bview("onehot", f32)
        ident_sb = _bview("ident", bf16)
        piw_sb = _bview("piw", f32)
        len_sb = _bview("len", f32)

        # ------- phase 0: before_score gathers (idle gpsimd, early);
        # their DVE consumers are emitted after phase 2 so the in-order
        # vector stream cannot stall phase 2 on gather completion.
        bgw = [None, None]
        for h in (1, 0):
            gw = gpool.tile([128, NIDX], f32, tag="bw", bufs=2,
                            name=f"bw{h}")
            nc.gpsimd.ap_gather(gw[:], bef[h][:].bitcast(f32), gidx[h],
                                channels=128, num_elems=IHALF // 2, d=1,
                                num_idxs=NIDX)
            bgw[h] = gw

        # ------- phase 1 epilogue -------
        h_f = pool.tile([128, D], f32)
        nc.scalar.activation(h_f[:], ps1[:, 0:D], Act.Tanh, scale=1.0 / WSCALE)
        h_bf = pool.tile([128, D + 1], bf16)
        nc.vector.tensor_copy(h_bf[:, 0:D], h_f[:])
        nc.vector.memset(h_bf[:, D:D + 1], 1.0)
        scr200 = pool.tile([128, D], f32)
        nc.vector.tensor_mul(scr200[:], h_f[:], ps1[:, D:2 * D])
        dot_p = pool.tile([128, 1], f32)
        nc.vector.tensor_reduce(dot_p[:], scr200[:], axis=mybir.AxisListType.X,
                                op=Alu.add)
        dot_sc = pool.tile([128, 1], f32)
        nc.vector.tensor_add(dot_sc[:], dot_p[:], ps1[:, 2 * D:2 * D + 1])
        dot_row = pool.tile([128, 1], f32)
        nc.vector.tensor_scalar_mul(dot_row[:], dot_sc[:], 1.0 / WSCALE)
        rsum = pool.tile([128, 1], f32)
        nc.vector.tensor_copy(rsum[:], ps1[:, 2 * D + 1:2 * D + 2])

        ps_hu = psum.tile([128, D], f32, tag="sm1")
        nc.tensor.matmul(ps_hu[:], onehot_sb, h_f[:], start=True, stop=True)
        hu_bf = pool.tile([128, D + 1], bf16)
        nc.vector.tensor_copy(hu_bf[:, 0:D], ps_hu[:])
        nc.vector.memset(hu_bf[:, D:D + 1], 1.0)

        # transposes: h^T and h_u^T, each augmented with a trailing ones row
        hta = pool.tile([128, BSH], f8)
        htb = pool.tile([73, BSH], f8)
        huta = pool.tile([128, UCAP], f8)
        hutb = pool.tile([73, UCAP], f8)
        for src, dsta, dstb in ((h_bf, hta, htb), (hu_bf, huta, hutb)):
            ps_t1 = psum.tile([128, 128], bf16, tag="sm2", bufs=2)
            nc.tensor.transpose(ps_t1[:], src[:, 0:128], ident_sb)
            nc.vector.tensor_copy(dsta[:], ps_t1[:])
            ps_t2 = psum.tile([73, 128], bf16, tag="sm2", bufs=2)
            nc.tensor.transpose(ps_t2[:], src[:, 128:D + 1], ident_sb)
            nc.vector.tensor_copy(dstb[:], ps_t2[:])

        # ------- phase 2: decoder stream, high item half first -------
        ulog = [None, None]
        ulog[1] = big.tile([UCAP, IHALF], bf16, tag="b1", name="ulog1")
        ulog[0] = big.tile([UCAP, IHALF], bf16, tag="b0", name="ulog0")
        ssum = pool.tile([128, NCH], f32)
        est = stream.tile([128, CH], f32, tag="est", bufs=2)
        for ci in range(NCH):
            c = ci + NCHH if ci < NCHH else ci - NCHH  # hi half first
            das = dwta[:, CH * c:CH * (c + 1)]
            dbs = dwtb[:, CH * c:CH * (c + 1)]
            pm = psmm.tile([128, CH], f32, tag="mm")
            nc.tensor.matmul(pm[:], hta[:], das, start=True, stop=False)
            nc.tensor.matmul(pm[:], htb[:], dbs, start=False, stop=True)
            pu = psmm.tile([128, CH], f32, tag="pu")
            nc.tensor.matmul(pu[:], huta[:], das, start=True, stop=False)
            nc.tensor.matmul(pu[:], hutb[:], dbs, start=False, stop=True)
            nc.scalar.activation(est[:], pm[:], Act.Exp,
                                 accum_out=ssum[:, c:c + 1],
                                 scale=1.0 / WSCALE)
            half = 1 if c >= NCHH else 0
            c0 = CH * (c - NCHH) if c >= NCHH else CH * c
            nc.vector.tensor_scalar_mul(ulog[half][:, c0:c0 + CH], pu[:],
                                        1.0 / WSCALE)

        # ------- phase 3a: before_score mask math (gathers done long ago) --
        s0 = pool.tile([128, 1], f32)
        s1 = pool.tile([128, 1], f32)
        w_t = [None, None]
        for h in (1, 0):
            val = parity_select(bgw[h], par[h], ipar[h], bf16, f"bv{h}")
            logb = gpool.tile([128, NIDX], f32, tag="tmp", bufs=2,
                              name=f"logb{h}")
            nc.scalar.activation(logb[:], val[:], Act.Ln)
            wt = pool.tile([128, NIDX], f32, name=f"w{h}")
            nc.vector.tensor_mul(wt[:], vm[h], val[:])
            w_t[h] = wt
            s0h = pool.tile([128, 1], f32, tag="s0h", bufs=2)
            nc.vector.tensor_reduce(s0h[:], wt[:], axis=mybir.AxisListType.X,
                                    op=Alu.add)
            scr_a = gpool.tile([128, NIDX], f32, tag="tmp", bufs=2,
                               name=f"wl{h}")
            nc.vector.tensor_mul(scr_a[:], wt[:], logb[:])
            s1h = pool.tile([128, 1], f32, tag="s1h", bufs=2)
            nc.vector.tensor_reduce(s1h[:], scr_a[:],
                                    axis=mybir.AxisListType.X, op=Alu.add)
            if h == 1:
                nc.vector.tensor_copy(s0[:], s0h[:])
                nc.vector.tensor_copy(s1[:], s1h[:])
            else:
                nc.vector.tensor_add(s0[:], s0[:], s0h[:])
                nc.vector.tensor_add(s1[:], s1[:], s1h[:])

        # ------- phase 3b: lse, user-logit gathers, combine -------
        s2 = pool.tile([128, 1], f32)
        for h in (1, 0):
            gw = gpool.tile([128, NIDX], f32, tag="tmp", bufs=2,
                            name=f"uw{h}")
            nc.gpsimd.ap_gather(gw[:], ulog[h][:].bitcast(f32), gidx[h],
                                channels=128, num_elems=IHALF // 2, d=1,
                                num_idxs=NIDX)
            valg = parity_select(gw, par[h], ipar[h], f32, f"uv{h}")
            scr = gpool.tile([128, NIDX], f32, tag="tmp", bufs=2,
                             name=f"ws{h}")
            nc.vector.tensor_mul(scr[:], w_t[h][:], valg[:])
            s2h = pool.tile([128, 1], f32, tag="s2h", bufs=2)
            nc.vector.tensor_reduce(s2h[:], scr[:], axis=mybir.AxisListType.X,
                                    op=Alu.add)
            if h == 1:
                nc.vector.tensor_copy(s2[:], s2h[:])
            else:
                nc.vector.tensor_add(s2[:], s2[:], s2h[:])

        s_tot = pool.tile([128, 1], f32)
        nc.vector.tensor_reduce(s_tot[:], ssum[:], axis=mybir.AxisListType.X,
                                op=Alu.add)
        lse = pool.tile([128, 1], f32)
        nc.scalar.activation(lse[:], s_tot[:], Act.Ln)

        ps_ls = psum.tile([128, 1], f32, tag="sm1")
        nc.tensor.matmul(ps_ls[:], onehot_sb, lse[:], start=True, stop=True)

        invlen = pool.tile([128, 1], f32)
        nc.vector.reciprocal(invlen[:], len_sb)

        # kl_slot = (s1 - s2 + lse_u*s0) * piw * invlen / U
        t0 = pool.tile([128, 1], f32)
        nc.vector.tensor_sub(t0[:], s1[:], s2[:])
        t1 = pool.tile([128, 1], f32)
        nc.vector.tensor_mul(t1[:], ps_ls[:], s0[:])
        t2 = pool.tile([128, 1], f32)
        nc.vector.tensor_add(t2[:], t0[:], t1[:])
        t3 = pool.tile([128, 1], f32)
        nc.vector.tensor_mul(t3[:], t2[:], piw_sb)
        t4 = pool.tile([128, 1], f32)
        nc.vector.tensor_mul(t4[:], t3[:], invlen[:])
        pair = pool.tile([128, 2], f32)
        nc.vector.tensor_scalar_mul(pair[:, 1:2], t4[:], 1.0 / U)

        # base_row = -(dot_row - lse*rsum)/B
        t5 = pool.tile([128, 1], f32)
        nc.vector.tensor_mul(t5[:], lse[:], rsum[:])
        t6 = pool.tile([128, 1], f32)
        nc.vector.tensor_sub(t6[:], dot_row[:], t5[:])
        nc.vector.tensor_scalar_mul(pair[:, 0:1], t6[:], -1.0 / B)

        ones1 = pool.tile([128, 1], f32)
        nc.vector.memset(ones1[:], 1.0)
        ps_fin = psum.tile([1, 2], f32, tag="sm2", bufs=2)
        nc.tensor.matmul(ps_fin[:], ones1[:], pair[:], start=True, stop=True)
        out_sb = pool.tile([1, 2], f32)
        nc.vector.tensor_copy(out_sb[:], ps_fin[:])
        nc.sync.dma_start(out=out_d[:], in_=out_sb[:])

    nc.compile()
    return nc


def get_program():
    if "nc" not in _prog_cache:
        _prog_cache["nc"] = _build_program()
    return _prog_cache["nc"]


def _pack_gather(users, inter_idx, lengths):
    """Ragged-pack per-group gather word indices, split at IHALF.

    Returns per half: wrapped int16 word-index array [128, NIDX//16],
    valid mask vm, parity mask par, inverse-parity mask ipar (all
    [128, NIDX]; ipar=1 at padding so the selected value stays > 0).
    """
    gidx = [np.zeros((128, NIDX // 16), np.int16) for _ in range(2)]
    vm = [np.zeros((128, NIDX), _BF16) for _ in range(2)]
    par = [np.zeros((128, NIDX), _BF16) for _ in range(2)]
    ipar = [np.ones((128, NIDX), _BF16) for _ in range(2)]
    jj = np.arange(NIDX)
    nu = len(users)
    slot_of = [(i % 8) * 16 + i // 8 for i in range(nu)]
    for g in range(8):
        members = [(slot_of[i] % 16, users[i]) for i in range(nu)
                   if slot_of[i] // 16 == g]
        lists = [np.zeros(NIDX, np.int64) for _ in range(2)]
        pos = [0, 0]
        for kk, u in members:
            il = inter_idx[u][:lengths[u]].astype(np.int64)
            for half, sel in enumerate((il < IHALF, il >= IHALF)):
                idx_h = il[sel] - half * IHALF
                n = len(idx_h)
                p0 = pos[half]
                assert p0 + n <= NIDX, "gather capacity exceeded"
                lists[half][p0:p0 + n] = idx_h >> 1
                rows = 16 * g + kk
                vm[half][rows, p0:p0 + n] = 1.0
                odd = (idx_h & 1).astype(_BF16)
                par[half][rows, p0:p0 + n] = odd
                # ipar defaults to 1 (padding-safe); overwrite real slots
                ipar[half][rows, p0:p0 + n] = 1.0 - odd.astype(np.float32)
                pos[half] += n
        for half in range(2):
            # positions are packed per group: every partition of the group
            # shares the same index list, wrapped across 16 partitions
            gidx[half][16 * g + (jj % 16), jj // 16] = lists[half][jj]
    # ipar rows for positions claimed by OTHER partitions in the group must
    # stay consistent with par: val is only consumed where vm=1, but keep
    # par+ipar <= 1 to avoid overflow concerns; nothing else needed.
    return gidx, vm, par, ipar


def make_in_maps(rating_vec, enc_w, enc_b, dec_w, dec_b, before_score, piw,
                 batch_idx, inter_idx, lengths):
    """Host-side sharding / layout prep. Index arithmetic + casts only."""
    f32 = np.float32
    rating_vec = np.asarray(rating_vec, f32)
    enc_w = np.asarray(enc_w, f32)
    enc_b = np.asarray(enc_b, f32)
    dec_w = np.asarray(dec_w, f32)
    dec_b = np.asarray(dec_b, f32)
    before_score = np.asarray(before_score, f32)
    piw = np.asarray(piw, f32)
    batch_idx = np.asarray(batch_idx)
    inter_idx = np.asarray(inter_idx)
    lengths = np.asarray(lengths)

    # shared (replicated) tensors; weights prescaled by WSCALE (power of
    # two, exactly undone on device) so fp8 encoding stays in normal range
    xmat = np.zeros((KT * 128, 402), f32)
    xmat[:I, 0:D] = enc_w.T
    xmat[:I, D:2 * D] = dec_w
    xmat[:I, 2 * D] = dec_b
    xmat[I, 0:D] = enc_b
    xmat[:, 0:2 * D + 1] *= WSCALE
    xmat[:I, 2 * D + 1] = 1.0
    xmat_bf = np.ascontiguousarray(
        xmat.astype(_F8).reshape(NG, KG, 128, 402)
        .transpose(0, 2, 1, 3)).view(np.uint8)

    dwt = dec_w.T * WSCALE  # [200, 20000]
    dwta = np.ascontiguousarray(dwt[:128]).astype(_F8).view(np.uint8)
    dwtb = np.concatenate([dwt[128:D], dec_b[None, :] * WSCALE],
                          axis=0).astype(_F8).view(np.uint8)

    ident = np.eye(128, dtype=_BF16)

    in_maps = []
    for c in range(NCORES):
        r0 = BSH * c
        ratingT = np.zeros((KT * 128, BSH), f32)
        ratingT[:I] = rating_vec[r0:r0 + BSH].T
        ratingT[I] = 1.0

        users = np.nonzero((batch_idx >= r0) & (batch_idx < r0 + BSH))[0]
        nu = len(users)
        assert nu <= UCAP, f"core {c}: {nu} users > capacity {UCAP}"

        slots = np.array([(i % 8) * 16 + i // 8 for i in range(nu)],
                         np.int64)
        bef = np.empty((UCAP, I), _BF16)
        bef[:] = before_score[0]
        bef[slots] = before_score[users]

        gidx, vm, par, ipar = _pack_gather(users, inter_idx, lengths)

        onehot_arr = np.zeros((128, UCAP), f32)
        onehot_arr[batch_idx[users] - r0, slots] = 1.0

        piw_arr = np.zeros((UCAP, 1), f32)
        piw_arr[slots, 0] = piw[users]
        len_arr = np.ones((UCAP, 1), f32)
        len_arr[slots, 0] = lengths[users].astype(f32)

        blob = np.zeros((128, BLOB_BYTES), np.uint8)

        def put(name, arr):
            off, sz = BLOB_LAYOUT[name]
            bview = np.ascontiguousarray(arr).view(np.uint8).reshape(128, sz)
            blob[:, off:off + sz] = bview

        put("vm0", vm[0]); put("vm1", vm[1])
        put("par0", par[0]); put("par1", par[1])
        put("ipar0", ipar[0]); put("ipar1", ipar[1])
        put("gidx0", gidx[0]); put("gidx1", gidx[1])
        put("onehot", onehot_arr)
        put("ident", ident)
        put("piw", piw_arr)
        put("len", len_arr)

        in_maps.append(dict(
            ratingT=np.ascontiguousarray(
                ratingT.astype(_F8).reshape(KT, 128, BSH)
                .transpose(1, 0, 2)).reshape(128, KT * BSH).view(np.uint8),
            xmat=xmat_bf,
            dwta=dwta,
            dwtb=dwtb,
            bef_0=np.ascontiguousarray(bef[:, :IHALF]),
            bef_1=np.ascontiguousarray(bef[:, IHALF:]),
            blob=blob,
        ))
    return in_maps


def combine(outs):
    base = f32sum(o[0, 0] for o in outs)
    kl = f32sum(o[0, 1] for o in outs)
    return np.float32(base), np.float32(kl)


def f32sum(it):
    acc = np.float32(0.0)
    for v in it:
        acc = np.float32(acc + np.float32(v))
    return acc


def kernel(**inputs):
    nc = get_program()
    in_maps = make_in_maps(**inputs)
    from concourse.bass_utils import run_bass_kernel_spmd
    res = run_bass_kernel_spmd(nc, in_maps, list(range(NCORES)))
    outs = [res.results[c]["out"] for c in range(NCORES)]
    return combine(outs)


# revision 42
# speedup vs baseline: 2.3185x; 1.0616x over previous
"""Trainium2 Bass kernel for nn_CL_VAE (Multi-VAE loss + contrastive-learning KL).

Strategy (8 NeuronCores, data-parallel over batch rows + common users):
  - core c owns batch rows [128c, 128c+128) and the common users whose
    batch_idx falls in that range (padded to 128 user slots, user = SBUF
    partition).
  - Host prep is layout-only: shard/transpose/concat/cast inputs, build
    index tables + 0/1 masks from the integer index inputs.
  - Device per core:
      phase 0: gather before_score at (user, inter_idx) positions with
               gpsimd ap_gather (ragged-packed word indices + parity
               select, split by item halves), w = b_sel at valid slots,
               S0 = sum w, S1 = sum w*log(b_sel).
      phase 1: one fused matmul over K = I(+1 bias row):
               [h_pre | rating@dec_w | rating@dec_b | sum(rating)] =
               rating_aug^T.T @ [enc_w^T | dec_w | dec_b | 1].
      phase 2: decoder matmuls logits = [h|1] @ [dec_w^T; dec_b] for both
               batch rows and user rows from resident weights, high item
               half first (so its user-logit gather overlaps the low
               half); exp+accumulate for softmax denominators.
      phase 3: lse = log(sum exp), gather user logits, S2 = sum(w*glog),
               combine into the two loss partials, reduce over partitions
               with a ones-matmul.
  - Host combines the 8 per-core [1,2] partials by summation only.

log-softmax identity used (exact in real arithmetic):
  sum_i log_probs*rating = sum_i logits*rating - lse*sum_i rating
  log(p_sel) = logits_sel - lse      (since p_sel = exp(log_probs))
The +EPS terms of the reference cancel in (log_b - log_p) and are below
fp32 resolution elsewhere.

Gathers run on bf16 data but ap_gather moves 4-byte words, so indices
are word indices (il>>1) and a host-provided parity mask pair selects
the bf16 half: val = ipar*even + par*odd.
"""

import numpy as np
import ml_dtypes

# ---- hardcoded problem shapes ----
B, I, D, U, L = 1024, 20000, 200, 512, 200
NCORES = 8
BSH = B // NCORES      # 128 batch rows per core
UCAP = 128             # padded user slots per core
KTOT = I + 1           # contraction length (+1 bias/ones row)
KG = 8                 # k-tiles per DMA batch in phase 1
KT = 160               # k-tiles (zero-padded up from 157)
NG = KT // KG          # 20 phase-1 DMA groups
CH = 500               # decoder free-dim chunk
NCH = I // CH          # 40
IHALF = I // 2         # item split for gather overlap (10000)
NCHH = NCH // 2        # 20 chunks per half
NIDX = 688             # ragged-packed gather indices per 16-part group/half

WSCALE = 32.0          # power-of-two weight prescale for fp8 encoding

# packed small-constant blob layout: per-partition byte (offset, size)
_B2 = NIDX * 2
BLOB_LAYOUT = {
    "vm0": (0, _B2), "vm1": (_B2, _B2),
    "par0": (2 * _B2, _B2), "par1": (3 * _B2, _B2),
    "ipar0": (4 * _B2, _B2), "ipar1": (5 * _B2, _B2),
    "gidx0": (6 * _B2, NIDX // 8), "gidx1": (6 * _B2 + NIDX // 8, NIDX // 8),
    "onehot": (6 * _B2 + NIDX // 4, 512),
    "ident": (6 * _B2 + NIDX // 4 + 512, 256),
    "piw": (6 * _B2 + NIDX // 4 + 768, 4),
    "len": (6 * _B2 + NIDX // 4 + 772, 4),
}
BLOB_BYTES = 6 * _B2 + NIDX // 4 + 776

_BF16 = ml_dtypes.bfloat16
_F8 = ml_dtypes.float8_e4m3

_prog_cache = {}


def _build_program():
    import concourse.bacc as bacc
    import concourse.mybir as mybir
    import concourse.tile as tile
    from contextlib import ExitStack

    f32 = mybir.dt.float32
    bf16 = mybir.dt.bfloat16
    i16 = mybir.dt.int16
    Act = mybir.ActivationFunctionType
    Alu = mybir.AluOpType

    nc = bacc.Bacc("TRN2", target_bir_lowering=False, debug=False,
                   num_devices=NCORES)

    f8 = mybir.dt.float8e4
    u8_ = mybir.dt.uint8
    ratingT = nc.dram_tensor("ratingT", [128, KT * BSH], u8_,
                             kind="ExternalInput")
    xmat = nc.dram_tensor("xmat", [NG, 128, KG, 402], u8_,
                          kind="ExternalInput")
    dwta_d = nc.dram_tensor("dwta", [128, I], u8_, kind="ExternalInput")
    dwtb_d = nc.dram_tensor("dwtb", [73, I], u8_, kind="ExternalInput")
    bef_d = [nc.dram_tensor(f"bef_{h}", [UCAP, IHALF], bf16,
                            kind="ExternalInput") for h in range(2)]
    u8 = mybir.dt.uint8
    blob_d = nc.dram_tensor("blob", [128, BLOB_BYTES], u8,
                            kind="ExternalInput")
    out_d = nc.dram_tensor("out", [1, 2], f32, kind="ExternalOutput")

    with ExitStack() as ctx:
        tc = ctx.enter_context(tile.TileContext(nc))
        pool = ctx.enter_context(tc.tile_pool(name="main", bufs=1))
        big = ctx.enter_context(tc.tile_pool(name="big", bufs=1))
        gpool = ctx.enter_context(tc.tile_pool(name="gat", bufs=1))
        stream = ctx.enter_context(tc.tile_pool(name="stream", bufs=3))
        psum = ctx.enter_context(tc.tile_pool(name="ps", bufs=1, space="PSUM"))
        psmm = ctx.enter_context(tc.tile_pool(name="psmm", bufs=2,
                                              space="PSUM"))

        def parity_select(gw, par_ap, ipar_ap, out_dtype, name):
            """val = ipar*even(gw) + par*odd(gw); returns [128, NIDX] tile."""
            wb = gw[:].bitcast(bf16).rearrange("p (j t) -> p j t", t=2)
            t_e = gpool.tile([128, NIDX], out_dtype, tag="val", bufs=2,
                             name=f"{name}_e")
            nc.vector.tensor_mul(t_e[:], ipar_ap, wb[:, :, 0:1])
            t_o = gpool.tile([128, NIDX], out_dtype, tag="val", bufs=2,
                             name=f"{name}_o")
            nc.vector.tensor_mul(t_o[:], par_ap, wb[:, :, 1:2])
            val = gpool.tile([128, NIDX], out_dtype, tag="val2", bufs=2,
                             name=f"{name}_v")
            nc.vector.tensor_add(val[:], t_e[:], t_o[:])
            return val

        # ------- phase 1 main matmul stream (DMA priority: first) -------
        rt_res = pool.tile([128, KT * BSH], mybir.dt.uint8)
        nc.scalar.dma_start(out=rt_res[:, :KT * BSH // 2],
                            in_=ratingT[:, :KT * BSH // 2])
        nc.scalar.dma_start(out=rt_res[:, KT * BSH // 2:],
                            in_=ratingT[:, KT * BSH // 2:])
        rt8 = rt_res[:].bitcast(f8)
        ps1 = psmm.tile([128, 402], f32, tag="mm")
        for g in range(NG):
            x_g = stream.tile([128, KG, 402], mybir.dt.uint8, tag="xk",
                              bufs=6)
            nc.sync.dma_start(out=x_g[:], in_=xmat[g])
            x8 = x_g[:].bitcast(f8)
            for kk in range(KG):
                k = KG * g + kk
                nc.tensor.matmul(ps1[:], rt8[:, BSH * k:BSH * (k + 1)],
                                 x8[:, kk, :],
                                 start=(k == 0), stop=(k == KT - 1))

        # ------- scalar-queue traffic: blob+bef early, then dwt -------
        blob_sb = pool.tile([128, BLOB_BYTES], mybir.dt.uint8)
        nc.scalar.dma_start(out=blob_sb[:], in_=blob_d[:])
        bef = [None, None]
        for h in (1, 0):
            bt = big.tile([UCAP, IHALF], bf16, tag=f"b{h}", name=f"bef{h}")
            nc.scalar.dma_start(out=bt[:], in_=bef_d[h][:])
            bef[h] = bt
        dwta_t = pool.tile([128, I], mybir.dt.uint8, name="dwta")
        dwtb_t = pool.tile([73, I], mybir.dt.uint8, name="dwtb")
        dwta = dwta_t[:].bitcast(f8)
        dwtb = dwtb_t[:].bitcast(f8)
        nc.scalar.dma_start(out=dwta_t[:, IHALF:I], in_=dwta_d[:, IHALF:I])
        nc.scalar.dma_start(out=dwtb_t[:, IHALF:I], in_=dwtb_d[:, IHALF:I])
        nc.scalar.dma_start(out=dwta_t[:, 0:IHALF], in_=dwta_d[:, 0:IHALF])
        nc.scalar.dma_start(out=dwtb_t[:, 0:IHALF], in_=dwtb_d[:, 0:IHALF])

        def _bview(name, dtype):
            off, sz = BLOB_LAYOUT[name]
            return blob_sb[:, off:off + sz].bitcast(dtype)

        vm = [_bview("vm0", bf16), _bview("vm1", bf16)]
        par = [_bview("par0", bf16), _bview("par1", bf16)]
        ipar = [_bview("ipar0", bf16), _bview("ipar1", bf16)]
        gidx = [_bview("gidx0", i16), _bview("gidx1", i16)]
        onehot_sb = _bview("onehot", f32)
        ident_sb = _bview("ident", bf16)
        piw_sb = _bview("piw", f32)
        len_sb = _bview("len", f32)

        # ------- phase 0: before_score gathers (idle gpsimd, early);
        # their DVE consumers are emitted after phase 2 so the in-order
        # vector stream cannot stall phase 2 on gather completion.
        bgw = [None, None]
        for h in (1, 0):
            gw = gpool.tile([128, NIDX], f32, tag="bw", bufs=2,
                            name=f"bw{h}")
            nc.gpsimd.ap_gather(gw[:], bef[h][:].bitcast(f32), gidx[h],
                                channels=128, num_elems=IHALF // 2, d=1,
                                num_idxs=NIDX)
            bgw[h] = gw

        # ------- phase 1 epilogue -------
        h_f = pool.tile([128, D], f32)
        nc.scalar.activation(h_f[:], ps1[:, 0:D], Act.Tanh, scale=1.0 / WSCALE)
        h_bf = pool.tile([128, D + 1], bf16)
        nc.vector.tensor_copy(h_bf[:, 0:D], h_f[:])
        nc.vector.memset(h_bf[:, D:D + 1], 1.0)
        scr200 = pool.tile([128, D], f32)
        nc.vector.tensor_mul(scr200[:], h_f[:], ps1[:, D:2 * D])
        dot_p = pool.tile([128, 1], f32)
        nc.vector.tensor_reduce(dot_p[:], scr200[:], axis=mybir.AxisListType.X,
                                op=Alu.add)
        dot_sc = pool.tile([128, 1], f32)
        nc.vector.tensor_add(dot_sc[:], dot_p[:], ps1[:, 2 * D:2 * D + 1])
        dot_row = pool.tile([128, 1], f32)
        nc.vector.tensor_scalar_mul(dot_row[:], dot_sc[:], 1.0 / WSCALE)
        rsum = pool.tile([128, 1], f32)
        nc.vector.tensor_copy(rsum[:], ps1[:, 2 * D + 1:2 * D + 2])

        ps_hu = psum.tile([128, D], f32, tag="sm1")
        nc.tensor.matmul(ps_hu[:], onehot_sb, h_f[:], start=True, stop=True)
        hu_bf = pool.tile([128, D + 1], bf16)
        nc.vector.tensor_copy(hu_bf[:, 0:D], ps_hu[:])
        nc.vector.memset(hu_bf[:, D:D + 1], 1.0)

        # transposes: h^T and h_u^T, each augmented with a trailing ones row
        hta = pool.tile([128, BSH], f8)
        htb = pool.tile([73, BSH], f8)
        huta = pool.tile([128, UCAP], f8)
        hutb = pool.tile([73, UCAP], f8)
        for src, dsta, dstb in ((h_bf, hta, htb), (hu_bf, huta, hutb)):
            ps_t1 = psum.tile([128, 128], bf16, tag="sm2", bufs=2)
            nc.tensor.transpose(ps_t1[:], src[:, 0:128], ident_sb)
            nc.vector.tensor_copy(dsta[:], ps_t1[:])
            ps_t2 = psum.tile([73, 128], bf16, tag="sm2", bufs=2)
            nc.tensor.transpose(ps_t2[:], src[:, 128:D + 1], ident_sb)
            nc.vector.tensor_copy(dstb[:], ps_t2[:])

        # ------- phase 2: decoder stream, high item half first -------
        ulog = [None, None]
        ulog[1] = big.tile([UCAP, IHALF], bf16, tag="b1", name="ulog1")
        ulog[0] = big.tile([UCAP, IHALF], bf16, tag="b0", name="ulog0")
        ssum = pool.tile([128, NCH], f32)
        est = stream.tile([128, CH], f32, tag="est", bufs=2)
        for ci in range(NCH):
            c = ci + NCHH if ci < NCHH else ci - NCHH  # hi half first
            das = dwta[:, CH * c:CH * (c + 1)]
            dbs = dwtb[:, CH * c:CH * (c + 1)]
            pm = psmm.tile([128, CH], f32, tag="mm")
            nc.tensor.matmul(pm[:], hta[:], das, start=True, stop=False)
            nc.tensor.matmul(pm[:], htb[:], dbs, start=False, stop=True)
            pu = psmm.tile([128, CH], f32, tag="pu")
            nc.tensor.matmul(pu[:], huta[:], das, start=True, stop=False)
            nc.tensor.matmul(pu[:], hutb[:], dbs, start=False, stop=True)
            nc.scalar.activation(est[:], pm[:], Act.Exp,
                                 accum_out=ssum[:, c:c + 1],
                                 scale=1.0 / WSCALE)
            half = 1 if c >= NCHH else 0
            c0 = CH * (c - NCHH) if c >= NCHH else CH * c
            nc.vector.tensor_scalar_mul(ulog[half][:, c0:c0 + CH], pu[:],
                                        1.0 / WSCALE)

        # ------- phase 3a: before_score mask math (gathers done long ago) --
        s0 = pool.tile([128, 1], f32)
        s1 = pool.tile([128, 1], f32)
        w_t = [None, None]
        for h in (1, 0):
            val = parity_select(bgw[h], par[h], ipar[h], bf16, f"bv{h}")
            logb = gpool.tile([128, NIDX], f32, tag="tmp", bufs=2,
                              name=f"logb{h}")
            nc.scalar.activation(logb[:], val[:], Act.Ln)
            wt = pool.tile([128, NIDX], f32, name=f"w{h}")
            nc.vector.tensor_mul(wt[:], vm[h], val[:])
            w_t[h] = wt
            s0h = pool.tile([128, 1], f32, tag="s0h", bufs=2)
            nc.vector.tensor_reduce(s0h[:], wt[:], axis=mybir.AxisListType.X,
                                    op=Alu.add)
            scr_a = gpool.tile([128, NIDX], f32, tag="tmp", bufs=2,
                               name=f"wl{h}")
            nc.vector.tensor_mul(scr_a[:], wt[:], logb[:])
            s1h = pool.tile([128, 1], f32, tag="s1h", bufs=2)
            nc.vector.tensor_reduce(s1h[:], scr_a[:],
                                    axis=mybir.AxisListType.X, op=Alu.add)
            if h == 1:
                nc.vector.tensor_copy(s0[:], s0h[:])
                nc.vector.tensor_copy(s1[:], s1h[:])
            else:
                nc.vector.tensor_add(s0[:], s0[:], s0h[:])
                nc.vector.tensor_add(s1[:], s1[:], s1h[:])

        # ------- phase 3b: lse, user-logit gathers, combine -------
        s2 = pool.tile([128, 1], f32)
        for h in (1, 0):
            gw = gpool.tile([128, NIDX], f32, tag="tmp", bufs=2,
                            name=f"uw{h}")
            nc.gpsimd.ap_gather(gw[:], ulog[h][:].bitcast(f32), gidx[h],
                                channels=128, num_elems=IHALF // 2, d=1,
                                num_idxs=NIDX)
            valg = parity_select(gw, par[h], ipar[h], f32, f"uv{h}")
            scr = gpool.tile([128, NIDX], f32, tag="tmp", bufs=2,
                             name=f"ws{h}")
            nc.vector.tensor_mul(scr[:], w_t[h][:], valg[:])
            s2h = pool.tile([128, 1], f32, tag="s2h", bufs=2)
            nc.vector.tensor_reduce(s2h[:], scr[:], axis=mybir.AxisListType.X,
                                    op=Alu.add)
            if h == 1:
                nc.vector.tensor_copy(s2[:], s2h[:])
            else:
                nc.vector.tensor_add(s2[:], s2[:], s2h[:])

        s_tot = pool.tile([128, 1], f32)
        nc.vector.tensor_reduce(s_tot[:], ssum[:], axis=mybir.AxisListType.X,
                                op=Alu.add)
        lse = pool.tile([128, 1], f32)
        nc.scalar.activation(lse[:], s_tot[:], Act.Ln)

        ps_ls = psum.tile([128, 1], f32, tag="sm1")
        nc.tensor.matmul(ps_ls[:], onehot_sb, lse[:], start=True, stop=True)

        invlen = pool.tile([128, 1], f32)
        nc.vector.reciprocal(invlen[:], len_sb)

        # kl_slot = (s1 - s2 + lse_u*s0) * piw * invlen / U
        t0 = pool.tile([128, 1], f32)
        nc.vector.tensor_sub(t0[:], s1[:], s2[:])
        t1 = pool.tile([128, 1], f32)
        nc.vector.tensor_mul(t1[:], ps_ls[:], s0[:])
        t2 = pool.tile([128, 1], f32)
        nc.vector.tensor_add(t2[:], t0[:], t1[:])
        t3 = pool.tile([128, 1], f32)
        nc.vector.tensor_mul(t3[:], t2[:], piw_sb)
        t4 = pool.tile([128, 1], f32)
        nc.vector.tensor_mul(t4[:], t3[:], invlen[:])
        pair = pool.tile([128, 2], f32)
        nc.vector.tensor_scalar_mul(pair[:, 1:2], t4[:], 1.0 / U)

        # base_row = -(dot_row - lse*rsum)/B
        t5 = pool.tile([128, 1], f32)
        nc.vector.tensor_mul(t5[:], lse[:], rsum[:])
        t6 = pool.tile([128, 1], f32)
        nc.vector.tensor_sub(t6[:], dot_row[:], t5[:])
        nc.vector.tensor_scalar_mul(pair[:, 0:1], t6[:], -1.0 / B)

        ones1 = pool.tile([128, 1], f32)
        nc.vector.memset(ones1[:], 1.0)
        ps_fin = psum.tile([1, 2], f32, tag="sm2", bufs=2)
        nc.tensor.matmul(ps_fin[:], ones1[:], pair[:], start=True, stop=True)
        out_sb = pool.tile([1, 2], f32)
        nc.vector.tensor_copy(out_sb[:], ps_fin[:])
        nc.sync.dma_start(out=out_d[:], in_=out_sb[:])

    nc.compile()
    return nc


def get_program():
    if "nc" not in _prog_cache:
        _prog_cache["nc"] = _build_program()
    return _prog_cache["nc"]


def _pack_gather(users, inter_idx, lengths):
    """Ragged-pack per-group gather word indices, split at IHALF.

    Returns per half: wrapped int16 word-index array [128, NIDX//16],
    valid mask vm, parity mask par, inverse-parity mask ipar (all
    [128, NIDX]; ipar=1 at padding so the selected value stays > 0).
    """
    gidx = [np.zeros((128, NIDX // 16), np.int16) for _ in range(2)]
    vm = [np.zeros((128, NIDX), _BF16) for _ in range(2)]
    par = [np.zeros((128, NIDX), _BF16) for _ in range(2)]
    ipar = [np.ones((128, NIDX), _BF16) for _ in range(2)]
    jj = np.arange(NIDX)
    nu = len(users)
    slot_of = [(i % 8) * 16 + i // 8 for i in range(nu)]
    for g in range(8):
        members = [(slot_of[i] % 16, users[i]) for i in range(nu)
                   if slot_of[i] // 16 == g]
        lists = [np.zeros(NIDX, np.int64) for _ in range(2)]
        pos = [0, 0]
        for kk, u in members:
            il = inter_idx[u][:lengths[u]].astype(np.int64)
            for half, sel in enumerate((il < IHALF, il >= IHALF)):
                idx_h = il[sel] - half * IHALF
                n = len(idx_h)
                p0 = pos[half]
                assert p0 + n <= NIDX, "gather capacity exceeded"
                lists[half][p0:p0 + n] = idx_h >> 1
                rows = 16 * g + kk
                vm[half][rows, p0:p0 + n] = 1.0
                odd = (idx_h & 1).astype(_BF16)
                par[half][rows, p0:p0 + n] = odd
                # ipar defaults to 1 (padding-safe); overwrite real slots
                ipar[half][rows, p0:p0 + n] = 1.0 - odd.astype(np.float32)
                pos[half] += n
        for half in range(2):
            # positions are packed per group: every partition of the group
            # shares the same index list, wrapped across 16 partitions
            gidx[half][16 * g + (jj % 16), jj // 16] = lists[half][jj]
    # ipar rows for positions claimed by OTHER partitions in the group must
    # stay consistent with par: val is only consumed where vm=1, but keep
    # par+ipar <= 1 to avoid overflow concerns; nothing else needed.
    return gidx, vm, par, ipar


def make_in_maps(rating_vec, enc_w, enc_b, dec_w, dec_b, before_score, piw,
                 batch_idx, inter_idx, lengths):
    """Host-side sharding / layout prep. Index arithmetic + casts only."""
    f32 = np.float32
    rating_vec = np.asarray(rating_vec, f32)
    enc_w = np.asarray(enc_w, f32)
    enc_b = np.asarray(enc_b, f32)
    dec_w = np.asarray(dec_w, f32)
    dec_b = np.asarray(dec_b, f32)
    before_score = np.asarray(before_score, f32)
    piw = np.asarray(piw, f32)
    batch_idx = np.asarray(batch_idx)
    inter_idx = np.asarray(inter_idx)
    lengths = np.asarray(lengths)

    # shared (replicated) tensors; weights prescaled by WSCALE (power of
    # two, exactly undone on device) so fp8 encoding stays in normal range
    xmat = np.zeros((KT * 128, 402), f32)
    xmat[:I, 0:D] = enc_w.T
    xmat[:I, D:2 * D] = dec_w
    xmat[:I, 2 * D] = dec_b
    xmat[I, 0:D] = enc_b
    xmat[:, 0:2 * D + 1] *= WSCALE
    xmat[:I, 2 * D + 1] = 1.0
    xmat_bf = np.ascontiguousarray(
        xmat.astype(_F8).reshape(NG, KG, 128, 402)
        .transpose(0, 2, 1, 3)).view(np.uint8)

    dwt = dec_w.T * WSCALE  # [200, 20000]
    dwta = np.ascontiguousarray(dwt[:128]).astype(_F8).view(np.uint8)
    dwtb = np.concatenate([dwt[128:D], dec_b[None, :] * WSCALE],
                          axis=0).astype(_F8).view(np.uint8)

    ident = np.eye(128, dtype=_BF16)

    in_maps = []
    for c in range(NCORES):
        r0 = BSH * c
        ratingT = np.zeros((KT * 128, BSH), f32)
        ratingT[:I] = rating_vec[r0:r0 + BSH].T
        ratingT[I] = 1.0

        users = np.nonzero((batch_idx >= r0) & (batch_idx < r0 + BSH))[0]
        nu = len(users)
        assert nu <= UCAP, f"core {c}: {nu} users > capacity {UCAP}"

        slots = np.array([(i % 8) * 16 + i // 8 for i in range(nu)],
                         np.int64)
        bef = np.empty((UCAP, I), _BF16)
        bef[:] = before_score[0]
        bef[slots] = before_score[users]

        gidx, vm, par, ipar = _pack_gather(users, inter_idx, lengths)

        onehot_arr = np.zeros((128, UCAP), f32)
        onehot_arr[batch_idx[users] - r0, slots] = 1.0

        piw_arr = np.zeros((UCAP, 1), f32)
        piw_arr[slots, 0] = piw[users]
        len_arr = np.ones((UCAP, 1), f32)
        len_arr[slots, 0] = lengths[users].astype(f32)

        blob = np.zeros((128, BLOB_BYTES), np.uint8)

        def put(name, arr):
            off, sz = BLOB_LAYOUT[name]
            bview = np.ascontiguousarray(arr).view(np.uint8).reshape(128, sz)
            blob[:, off:off + sz] = bview

        put("vm0", vm[0]); put("vm1", vm[1])
        put("par0", par[0]); put("par1", par[1])
        put("ipar0", ipar[0]); put("ipar1", ipar[1])
        put("gidx0", gidx[0]); put("gidx1", gidx[1])
        put("onehot", onehot_arr)
        put("ident", ident)
        put("piw", piw_arr)
        put("len", len_arr)

        in_maps.append(dict(
            ratingT=np.ascontiguousarray(
                ratingT.astype(_F8).reshape(KT, 128, BSH)
                .transpose(1, 0, 2)).reshape(128, KT * BSH).view(np.uint8),
            xmat=xmat_bf,
            dwta=dwta,
            dwtb=dwtb,
            bef_0=np.ascontiguousarray(bef[:, :IHALF]),
            bef_1=np.ascontiguousarray(bef[:, IHALF:]),
            blob=blob,
        ))
    return in_maps


def combine(outs):
    base = f32sum(o[0, 0] for o in outs)
    kl = f32sum(o[0, 1] for o in outs)
    return np.float32(base), np.float32(kl)


def f32sum(it):
    acc = np.float32(0.0)
    for v in it:
        acc = np.float32(acc + np.float32(v))
    return acc


def kernel(**inputs):
    nc = get_program()
    in_maps = make_in_maps(**inputs)
    from concourse.bass_utils import run_bass_kernel_spmd
    res = run_bass_kernel_spmd(nc, in_maps, list(range(NCORES)))
    outs = [res.results[c]["out"] for c in range(NCORES)]
    return combine(outs)


# revision 43
# speedup vs baseline: 2.3938x; 1.0325x over previous
"""Trainium2 Bass kernel for nn_CL_VAE (Multi-VAE loss + contrastive-learning KL).

Strategy (8 NeuronCores, data-parallel over batch rows + common users):
  - core c owns batch rows [128c, 128c+128) and the common users whose
    batch_idx falls in that range (padded to 128 user slots, user = SBUF
    partition).
  - Host prep is layout-only: shard/transpose/concat/cast inputs, build
    index tables + 0/1 masks from the integer index inputs.
  - Device per core:
      phase 0: gather before_score at (user, inter_idx) positions with
               gpsimd ap_gather (ragged-packed word indices + parity
               select, split by item halves), w = b_sel at valid slots,
               S0 = sum w, S1 = sum w*log(b_sel).
      phase 1: one fused matmul over K = I(+1 bias row):
               [h_pre | rating@dec_w | rating@dec_b | sum(rating)] =
               rating_aug^T.T @ [enc_w^T | dec_w | dec_b | 1].
      phase 2: decoder matmuls logits = [h|1] @ [dec_w^T; dec_b] for both
               batch rows and user rows from resident weights, high item
               half first (so its user-logit gather overlaps the low
               half); exp+accumulate for softmax denominators.
      phase 3: lse = log(sum exp), gather user logits, S2 = sum(w*glog),
               combine into the two loss partials, reduce over partitions
               with a ones-matmul.
  - Host combines the 8 per-core [1,2] partials by summation only.

log-softmax identity used (exact in real arithmetic):
  sum_i log_probs*rating = sum_i logits*rating - lse*sum_i rating
  log(p_sel) = logits_sel - lse      (since p_sel = exp(log_probs))
The +EPS terms of the reference cancel in (log_b - log_p) and are below
fp32 resolution elsewhere.

Gathers run on bf16 data but ap_gather moves 4-byte words, so indices
are word indices (il>>1) and a host-provided parity mask pair selects
the bf16 half: val = ipar*even + par*odd.
"""

import numpy as np
import ml_dtypes

# ---- hardcoded problem shapes ----
B, I, D, U, L = 1024, 20000, 200, 512, 200
NCORES = 8
BSH = B // NCORES      # 128 batch rows per core
UCAP = 128             # padded user slots per core
KTOT = I + 1           # contraction length (+1 bias/ones row)
KG = 8                 # k-tiles per DMA batch in phase 1
KT = 160               # k-tiles (zero-padded up from 157)
NG = KT // KG          # 20 phase-1 DMA groups
CH = 500               # decoder free-dim chunk
NCH = I // CH          # 40
IHALF = I // 2         # item split for gather overlap (10000)
NCHH = NCH // 2        # 20 chunks per half
NIDX = 704             # ragged-packed gather indices per 16-part group/half

WSCALE = 32.0          # power-of-two weight prescale for fp8 encoding

# packed small-constant blob layout: per-partition byte (offset, size)
_B2 = NIDX * 2
BLOB_LAYOUT = {
    "vm0": (0, _B2), "vm1": (_B2, _B2),
    "par0": (2 * _B2, _B2), "par1": (3 * _B2, _B2),
    "ipar0": (4 * _B2, _B2), "ipar1": (5 * _B2, _B2),
    "gidx0": (6 * _B2, NIDX // 8), "gidx1": (6 * _B2 + NIDX // 8, NIDX // 8),
    "onehot": (6 * _B2 + NIDX // 4, 512),
    "ident": (6 * _B2 + NIDX // 4 + 512, 256),
    "piw": (6 * _B2 + NIDX // 4 + 768, 4),
    "len": (6 * _B2 + NIDX // 4 + 772, 4),
}
BLOB_BYTES = 6 * _B2 + NIDX // 4 + 776

_BF16 = ml_dtypes.bfloat16
_F8 = ml_dtypes.float8_e4m3

_prog_cache = {}


def _build_program():
    import concourse.bacc as bacc
    import concourse.mybir as mybir
    import concourse.tile as tile
    from contextlib import ExitStack

    f32 = mybir.dt.float32
    bf16 = mybir.dt.bfloat16
    i16 = mybir.dt.int16
    Act = mybir.ActivationFunctionType
    Alu = mybir.AluOpType

    nc = bacc.Bacc("TRN2", target_bir_lowering=False, debug=False,
                   num_devices=NCORES)

    f8 = mybir.dt.float8e4
    u8_ = mybir.dt.uint8
    ratingT = nc.dram_tensor("ratingT", [128, KT * BSH], u8_,
                             kind="ExternalInput")
    xmat = nc.dram_tensor("xmat", [NG, 128, KG, 402], u8_,
                          kind="ExternalInput")
    dwta_d = nc.dram_tensor("dwta", [128, I], u8_, kind="ExternalInput")
    dwtb_d = nc.dram_tensor("dwtb", [73, I], u8_, kind="ExternalInput")
    bef_d = [nc.dram_tensor(f"bef_{h}", [UCAP, IHALF], bf16,
                            kind="ExternalInput") for h in range(2)]
    u8 = mybir.dt.uint8
    blob_d = nc.dram_tensor("blob", [128, BLOB_BYTES], u8,
                            kind="ExternalInput")
    out_d = nc.dram_tensor("out", [1, 2], f32, kind="ExternalOutput")

    with ExitStack() as ctx:
        tc = ctx.enter_context(tile.TileContext(nc))
        pool = ctx.enter_context(tc.tile_pool(name="main", bufs=1))
        big = ctx.enter_context(tc.tile_pool(name="big", bufs=1))
        gpool = ctx.enter_context(tc.tile_pool(name="gat", bufs=1))
        stream = ctx.enter_context(tc.tile_pool(name="stream", bufs=3))
        psum = ctx.enter_context(tc.tile_pool(name="ps", bufs=1, space="PSUM"))
        psmm = ctx.enter_context(tc.tile_pool(name="psmm", bufs=2,
                                              space="PSUM"))

        def parity_select(gw, par_ap, ipar_ap, out_dtype, name):
            """val = ipar*even(gw) + par*odd(gw); returns [128, NIDX] tile."""
            wb = gw[:].bitcast(bf16).rearrange("p (j t) -> p j t", t=2)
            t_e = gpool.tile([128, NIDX], out_dtype, tag="val", bufs=2,
                             name=f"{name}_e")
            nc.vector.tensor_mul(t_e[:], ipar_ap, wb[:, :, 0:1])
            t_o = gpool.tile([128, NIDX], out_dtype, tag="val", bufs=2,
                             name=f"{name}_o")
            nc.vector.tensor_mul(t_o[:], par_ap, wb[:, :, 1:2])
            val = gpool.tile([128, NIDX], out_dtype, tag="val2", bufs=2,
                             name=f"{name}_v")
            nc.vector.tensor_add(val[:], t_e[:], t_o[:])
            return val

        # ------- phase 1 main matmul stream (DMA priority: first) -------
        rt_res = pool.tile([128, KT * BSH], mybir.dt.uint8)
        nc.scalar.dma_start(out=rt_res[:, :KT * BSH // 2],
                            in_=ratingT[:, :KT * BSH // 2])
        nc.scalar.dma_start(out=rt_res[:, KT * BSH // 2:],
                            in_=ratingT[:, KT * BSH // 2:])
        rt8 = rt_res[:].bitcast(f8)
        ps1 = psmm.tile([128, 402], f32, tag="mm")
        for g in range(NG):
            x_g = stream.tile([128, KG, 402], mybir.dt.uint8, tag="xk",
                              bufs=6)
            nc.sync.dma_start(out=x_g[:], in_=xmat[g])
            x8 = x_g[:].bitcast(f8)
            for kk in range(KG):
                k = KG * g + kk
                nc.tensor.matmul(ps1[:], rt8[:, BSH * k:BSH * (k + 1)],
                                 x8[:, kk, :],
                                 start=(k == 0), stop=(k == KT - 1))

        # ------- scalar-queue traffic: blob+bef early, then dwt -------
        blob_sb = pool.tile([128, BLOB_BYTES], mybir.dt.uint8)
        nc.scalar.dma_start(out=blob_sb[:], in_=blob_d[:])
        bef = [None, None]
        for h in (1, 0):
            bt = big.tile([UCAP, IHALF], bf16, tag=f"b{h}", name=f"bef{h}")
            nc.scalar.dma_start(out=bt[:], in_=bef_d[h][:])
            bef[h] = bt
        dwta_t = pool.tile([128, I], mybir.dt.uint8, name="dwta")
        dwtb_t = pool.tile([73, I], mybir.dt.uint8, name="dwtb")
        dwta = dwta_t[:].bitcast(f8)
        dwtb = dwtb_t[:].bitcast(f8)
        nc.scalar.dma_start(out=dwta_t[:, IHALF:I], in_=dwta_d[:, IHALF:I])
        nc.scalar.dma_start(out=dwtb_t[:, IHALF:I], in_=dwtb_d[:, IHALF:I])
        nc.scalar.dma_start(out=dwta_t[:, 0:IHALF], in_=dwta_d[:, 0:IHALF])
        nc.scalar.dma_start(out=dwtb_t[:, 0:IHALF], in_=dwtb_d[:, 0:IHALF])

        def _bview(name, dtype):
            off, sz = BLOB_LAYOUT[name]
            return blob_sb[:, off:off + sz].bitcast(dtype)

        vm = [_bview("vm0", bf16), _bview("vm1", bf16)]
        par = [_bview("par0", bf16), _bview("par1", bf16)]
        ipar = [_bview("ipar0", bf16), _bview("ipar1", bf16)]
        gidx = [_bview("gidx0", i16), _bview("gidx1", i16)]
        onehot_sb = _bview("onehot", f32)
        ident_sb = _bview("ident", bf16)
        piw_sb = _bview("piw", f32)
        len_sb = _bview("len", f32)

        # ------- phase 0: before_score gathers (idle gpsimd, early);
        # their DVE consumers are emitted after phase 2 so the in-order
        # vector stream cannot stall phase 2 on gather completion.
        bgw = [None, None]
        for h in (1, 0):
            gw = gpool.tile([128, NIDX], f32, tag="bw", bufs=2,
                            name=f"bw{h}")
            nc.gpsimd.ap_gather(gw[:], bef[h][:].bitcast(f32), gidx[h],
                                channels=128, num_elems=IHALF // 2, d=1,
                                num_idxs=NIDX)
            bgw[h] = gw

        # ------- phase 1 epilogue -------
        h_f = pool.tile([128, D], f32)
        nc.scalar.activation(h_f[:], ps1[:, 0:D], Act.Tanh, scale=1.0 / WSCALE)
        h_bf = pool.tile([128, D + 1], bf16)
        nc.vector.tensor_copy(h_bf[:, 0:D], h_f[:])
        nc.vector.memset(h_bf[:, D:D + 1], 1.0)
        scr200 = pool.tile([128, D], f32)
        nc.vector.tensor_mul(scr200[:], h_f[:], ps1[:, D:2 * D])
        dot_p = pool.tile([128, 1], f32)
        nc.vector.tensor_reduce(dot_p[:], scr200[:], axis=mybir.AxisListType.X,
                                op=Alu.add)
        dot_sc = pool.tile([128, 1], f32)
        nc.vector.tensor_add(dot_sc[:], dot_p[:], ps1[:, 2 * D:2 * D + 1])
        dot_row = pool.tile([128, 1], f32)
        nc.vector.tensor_scalar_mul(dot_row[:], dot_sc[:], 1.0 / WSCALE)
        rsum = pool.tile([128, 1], f32)
        nc.vector.tensor_copy(rsum[:], ps1[:, 2 * D + 1:2 * D + 2])

        ps_hu = psum.tile([128, D], f32, tag="sm1")
        nc.tensor.matmul(ps_hu[:], onehot_sb, h_f[:], start=True, stop=True)
        hu_bf = pool.tile([128, D + 1], bf16)
        nc.vector.tensor_copy(hu_bf[:, 0:D], ps_hu[:])
        nc.vector.memset(hu_bf[:, D:D + 1], 1.0)

        # transposes: h^T and h_u^T, each augmented with a trailing ones row
        hta = pool.tile([128, BSH], f8)
        htb = pool.tile([73, BSH], f8)
        huta = pool.tile([128, UCAP], f8)
        hutb = pool.tile([73, UCAP], f8)
        for src, dsta, dstb in ((h_bf, hta, htb), (hu_bf, huta, hutb)):
            ps_t1 = psum.tile([128, 128], bf16, tag="sm2", bufs=2)
            nc.tensor.transpose(ps_t1[:], src[:, 0:128], ident_sb)
            nc.vector.tensor_copy(dsta[:], ps_t1[:])
            ps_t2 = psum.tile([73, 128], bf16, tag="sm2", bufs=2)
            nc.tensor.transpose(ps_t2[:], src[:, 128:D + 1], ident_sb)
            nc.vector.tensor_copy(dstb[:], ps_t2[:])

        # ------- phase 2: decoder stream, high item half first -------
        ulog = [None, None]
        ulog[1] = big.tile([UCAP, IHALF], bf16, tag="b1", name="ulog1")
        ulog[0] = big.tile([UCAP, IHALF], bf16, tag="b0", name="ulog0")
        ssum = pool.tile([128, NCH], f32)
        est = stream.tile([128, CH], f32, tag="est", bufs=2)
        for ci in range(NCH):
            c = ci + NCHH if ci < NCHH else ci - NCHH  # hi half first
            das = dwta[:, CH * c:CH * (c + 1)]
            dbs = dwtb[:, CH * c:CH * (c + 1)]
            pm = psmm.tile([128, CH], f32, tag="mm")
            nc.tensor.matmul(pm[:], hta[:], das, start=True, stop=False)
            nc.tensor.matmul(pm[:], htb[:], dbs, start=False, stop=True)
            pu = psmm.tile([128, CH], f32, tag="pu")
            nc.tensor.matmul(pu[:], huta[:], das, start=True, stop=False)
            nc.tensor.matmul(pu[:], hutb[:], dbs, start=False, stop=True)
            nc.scalar.activation(est[:], pm[:], Act.Exp,
                                 accum_out=ssum[:, c:c + 1],
                                 scale=1.0 / WSCALE)
            half = 1 if c >= NCHH else 0
            c0 = CH * (c - NCHH) if c >= NCHH else CH * c
            nc.vector.tensor_scalar_mul(ulog[half][:, c0:c0 + CH], pu[:],
                                        1.0 / WSCALE)

        # ------- phase 3a: before_score mask math (gathers done long ago) --
        s0 = pool.tile([128, 1], f32)
        s1 = pool.tile([128, 1], f32)
        w_t = [None, None]
        for h in (1, 0):
            val = parity_select(bgw[h], par[h], ipar[h], bf16, f"bv{h}")
            logb = gpool.tile([128, NIDX], f32, tag="tmp", bufs=2,
                              name=f"logb{h}")
            nc.scalar.activation(logb[:], val[:], Act.Ln)
            wt = pool.tile([128, NIDX], f32, name=f"w{h}")
            nc.vector.tensor_mul(wt[:], vm[h], val[:])
            w_t[h] = wt
            s0h = pool.tile([128, 1], f32, tag="s0h", bufs=2)
            nc.vector.tensor_reduce(s0h[:], wt[:], axis=mybir.AxisListType.X,
                                    op=Alu.add)
            scr_a = gpool.tile([128, NIDX], f32, tag="tmp", bufs=2,
                               name=f"wl{h}")
            nc.vector.tensor_mul(scr_a[:], wt[:], logb[:])
            s1h = pool.tile([128, 1], f32, tag="s1h", bufs=2)
            nc.vector.tensor_reduce(s1h[:], scr_a[:],
                                    axis=mybir.AxisListType.X, op=Alu.add)
            if h == 1:
                nc.vector.tensor_copy(s0[:], s0h[:])
                nc.vector.tensor_copy(s1[:], s1h[:])
            else:
                nc.vector.tensor_add(s0[:], s0[:], s0h[:])
                nc.vector.tensor_add(s1[:], s1[:], s1h[:])

        # ------- phase 3b: lse, user-logit gathers, combine -------
        s2 = pool.tile([128, 1], f32)
        for h in (1, 0):
            gw = gpool.tile([128, NIDX], f32, tag="tmp", bufs=2,
                            name=f"uw{h}")
            nc.gpsimd.ap_gather(gw[:], ulog[h][:].bitcast(f32), gidx[h],
                                channels=128, num_elems=IHALF // 2, d=1,
                                num_idxs=NIDX)
            valg = parity_select(gw, par[h], ipar[h], f32, f"uv{h}")
            scr = gpool.tile([128, NIDX], f32, tag="tmp", bufs=2,
                             name=f"ws{h}")
            nc.vector.tensor_mul(scr[:], w_t[h][:], valg[:])
            s2h = pool.tile([128, 1], f32, tag="s2h", bufs=2)
            nc.vector.tensor_reduce(s2h[:], scr[:], axis=mybir.AxisListType.X,
                                    op=Alu.add)
            if h == 1:
                nc.vector.tensor_copy(s2[:], s2h[:])
            else:
                nc.vector.tensor_add(s2[:], s2[:], s2h[:])

        s_tot = pool.tile([128, 1], f32)
        nc.vector.tensor_reduce(s_tot[:], ssum[:], axis=mybir.AxisListType.X,
                                op=Alu.add)
        lse = pool.tile([128, 1], f32)
        nc.scalar.activation(lse[:], s_tot[:], Act.Ln)

        ps_ls = psum.tile([128, 1], f32, tag="sm1")
        nc.tensor.matmul(ps_ls[:], onehot_sb, lse[:], start=True, stop=True)

        invlen = pool.tile([128, 1], f32)
        nc.vector.reciprocal(invlen[:], len_sb)

        # kl_slot = (s1 - s2 + lse_u*s0) * piw * invlen / U
        t0 = pool.tile([128, 1], f32)
        nc.vector.tensor_sub(t0[:], s1[:], s2[:])
        t1 = pool.tile([128, 1], f32)
        nc.vector.tensor_mul(t1[:], ps_ls[:], s0[:])
        t2 = pool.tile([128, 1], f32)
        nc.vector.tensor_add(t2[:], t0[:], t1[:])
        t3 = pool.tile([128, 1], f32)
        nc.vector.tensor_mul(t3[:], t2[:], piw_sb)
        t4 = pool.tile([128, 1], f32)
        nc.vector.tensor_mul(t4[:], t3[:], invlen[:])
        pair = pool.tile([128, 2], f32)
        nc.vector.tensor_scalar_mul(pair[:, 1:2], t4[:], 1.0 / U)

        # base_row = -(dot_row - lse*rsum)/B
        t5 = pool.tile([128, 1], f32)
        nc.vector.tensor_mul(t5[:], lse[:], rsum[:])
        t6 = pool.tile([128, 1], f32)
        nc.vector.tensor_sub(t6[:], dot_row[:], t5[:])
        nc.vector.tensor_scalar_mul(pair[:, 0:1], t6[:], -1.0 / B)

        ones1 = pool.tile([128, 1], f32)
        nc.vector.memset(ones1[:], 1.0)
        ps_fin = psum.tile([1, 2], f32, tag="sm2", bufs=2)
        nc.tensor.matmul(ps_fin[:], ones1[:], pair[:], start=True, stop=True)
        out_sb = pool.tile([1, 2], f32)
        nc.vector.tensor_copy(out_sb[:], ps_fin[:])
        nc.sync.dma_start(out=out_d[:], in_=out_sb[:])

    nc.compile()
    return nc


def get_program():
    if "nc" not in _prog_cache:
        _prog_cache["nc"] = _build_program()
    return _prog_cache["nc"]


def _pack_gather(users, inter_idx, lengths):
    """Ragged-pack per-group gather word indices, split at IHALF.

    Returns per half: wrapped int16 word-index array [128, NIDX//16],
    valid mask vm, parity mask par, inverse-parity mask ipar (all
    [128, NIDX]; ipar=1 at padding so the selected value stays > 0).
    """
    gidx = [np.zeros((128, NIDX // 16), np.int16) for _ in range(2)]
    vm = [np.zeros((128, NIDX), _BF16) for _ in range(2)]
    par = [np.zeros((128, NIDX), _BF16) for _ in range(2)]
    ipar = [np.ones((128, NIDX), _BF16) for _ in range(2)]
    jj = np.arange(NIDX)
    nu = len(users)
    slot_of = [(i % 8) * 16 + i // 8 for i in range(nu)]
    for g in range(8):
        members = [(slot_of[i] % 16, users[i]) for i in range(nu)
                   if slot_of[i] // 16 == g]
        lists = [np.zeros(NIDX, np.int64) for _ in range(2)]
        pos = [0, 0]
        for kk, u in members:
            il = inter_idx[u][:lengths[u]].astype(np.int64)
            for half, sel in enumerate((il < IHALF, il >= IHALF)):
                idx_h = il[sel] - half * IHALF
                n = len(idx_h)
                p0 = pos[half]
                assert p0 + n <= NIDX, "gather capacity exceeded"
                lists[half][p0:p0 + n] = idx_h >> 1
                rows = 16 * g + kk
                vm[half][rows, p0:p0 + n] = 1.0
                odd = (idx_h & 1).astype(_BF16)
                par[half][rows, p0:p0 + n] = odd
                # ipar defaults to 1 (padding-safe); overwrite real slots
                ipar[half][rows, p0:p0 + n] = 1.0 - odd.astype(np.float32)
                pos[half] += n
        for half in range(2):
            # positions are packed per group: every partition of the group
            # shares the same index list, wrapped across 16 partitions
            gidx[half][16 * g + (jj % 16), jj // 16] = lists[half][jj]
    # ipar rows for positions claimed by OTHER partitions in the group must
    # stay consistent with par: val is only consumed where vm=1, but keep
    # par+ipar <= 1 to avoid overflow concerns; nothing else needed.
    return gidx, vm, par, ipar


def make_in_maps(rating_vec, enc_w, enc_b, dec_w, dec_b, before_score, piw,
                 batch_idx, inter_idx, lengths):
    """Host-side sharding / layout prep. Index arithmetic + casts only."""
    f32 = np.float32
    rating_vec = np.asarray(rating_vec, f32)
    enc_w = np.asarray(enc_w, f32)
    enc_b = np.asarray(enc_b, f32)
    dec_w = np.asarray(dec_w, f32)
    dec_b = np.asarray(dec_b, f32)
    before_score = np.asarray(before_score, f32)
    piw = np.asarray(piw, f32)
    batch_idx = np.asarray(batch_idx)
    inter_idx = np.asarray(inter_idx)
    lengths = np.asarray(lengths)

    # shared (replicated) tensors; weights prescaled by WSCALE (power of
    # two, exactly undone on device) so fp8 encoding stays in normal range
    xmat = np.zeros((KT * 128, 402), f32)
    xmat[:I, 0:D] = enc_w.T
    xmat[:I, D:2 * D] = dec_w
    xmat[:I, 2 * D] = dec_b
    xmat[I, 0:D] = enc_b
    xmat[:, 0:2 * D + 1] *= WSCALE
    xmat[:I, 2 * D + 1] = 1.0
    xmat_bf = np.ascontiguousarray(
        xmat.astype(_F8).reshape(NG, KG, 128, 402)
        .transpose(0, 2, 1, 3)).view(np.uint8)

    dwt = dec_w.T * WSCALE  # [200, 20000]
    dwta = np.ascontiguousarray(dwt[:128]).astype(_F8).view(np.uint8)
    dwtb = np.concatenate([dwt[128:D], dec_b[None, :] * WSCALE],
                          axis=0).astype(_F8).view(np.uint8)

    ident = np.eye(128, dtype=_BF16)

    in_maps = []
    for c in range(NCORES):
        r0 = BSH * c
        ratingT = np.zeros((KT * 128, BSH), f32)
        ratingT[:I] = rating_vec[r0:r0 + BSH].T
        ratingT[I] = 1.0

        users = np.nonzero((batch_idx >= r0) & (batch_idx < r0 + BSH))[0]
        nu = len(users)
        assert nu <= UCAP, f"core {c}: {nu} users > capacity {UCAP}"

        slots = np.array([(i % 8) * 16 + i // 8 for i in range(nu)],
                         np.int64)
        bef = np.empty((UCAP, I), _BF16)
        bef[:] = before_score[0]
        bef[slots] = before_score[users]

        gidx, vm, par, ipar = _pack_gather(users, inter_idx, lengths)

        onehot_arr = np.zeros((128, UCAP), f32)
        onehot_arr[batch_idx[users] - r0, slots] = 1.0

        piw_arr = np.zeros((UCAP, 1), f32)
        piw_arr[slots, 0] = piw[users]
        len_arr = np.ones((UCAP, 1), f32)
        len_arr[slots, 0] = lengths[users].astype(f32)

        blob = np.zeros((128, BLOB_BYTES), np.uint8)

        def put(name, arr):
            off, sz = BLOB_LAYOUT[name]
            bview = np.ascontiguousarray(arr).view(np.uint8).reshape(128, sz)
            blob[:, off:off + sz] = bview

        put("vm0", vm[0]); put("vm1", vm[1])
        put("par0", par[0]); put("par1", par[1])
        put("ipar0", ipar[0]); put("ipar1", ipar[1])
        put("gidx0", gidx[0]); put("gidx1", gidx[1])
        put("onehot", onehot_arr)
        put("ident", ident)
        put("piw", piw_arr)
        put("len", len_arr)

        in_maps.append(dict(
            ratingT=np.ascontiguousarray(
                ratingT.astype(_F8).reshape(KT, 128, BSH)
                .transpose(1, 0, 2)).reshape(128, KT * BSH).view(np.uint8),
            xmat=xmat_bf,
            dwta=dwta,
            dwtb=dwtb,
            bef_0=np.ascontiguousarray(bef[:, :IHALF]),
            bef_1=np.ascontiguousarray(bef[:, IHALF:]),
            blob=blob,
        ))
    return in_maps


def combine(outs):
    base = f32sum(o[0, 0] for o in outs)
    kl = f32sum(o[0, 1] for o in outs)
    return np.float32(base), np.float32(kl)


def f32sum(it):
    acc = np.float32(0.0)
    for v in it:
        acc = np.float32(acc + np.float32(v))
    return acc


def kernel(**inputs):
    nc = get_program()
    in_maps = make_in_maps(**inputs)
    from concourse.bass_utils import run_bass_kernel_spmd
    res = run_bass_kernel_spmd(nc, in_maps, list(range(NCORES)))
    outs = [res.results[c]["out"] for c in range(NCORES)]
    return combine(outs)
